# revision 53
# baseline (speedup 1.0000x reference)
"""Block-sparse multi-head attention on 8 Trainium2 NeuronCores.

Problem: y = proj(softmax(mask(q @ k^T / sqrt(hd))) @ v) for
B=2, S=2048, D=1024, H=16 heads, block size 128, with a [16,16] boolean
block mask (True = masked) applied to strictly-upper (k-block > q-block)
blocks.

Sharding: batch x head-group. Core c handles batch c//4 and heads
[4*(c%4), 4*(c%4)+4). No collectives: the host pre-slices inputs
(including pre-transposing x to x^T) and sums the 4 per-batch partial
projection outputs on the way out.

This version fuses all phases into one software-pipelined instruction
stream to keep ScalarE (the exp bottleneck, ~100us/core) and the PE
(~123us/core) simultaneously busy:
  - x/w_qkv/w_v are uploaded in bf16 (halves input DMA to ~6.5MB);
    DMAs are chunked by xT column-slice and issued in consumption order
    so the first attention exp lands ~8us into the kernel.
  - qk-gen for head pair 0 runs first; v-gen and pair-1 qk-gen chunks
    are interleaved into the attention pipeline of heads 1 and 0
    (sharing one PSUM ring) so the PE never idles long enough for HAM
    to re-throttle the clock.
  - attention per head runs as two window passes g=0/1 (pa [65,1024]
    PSUM x2-ring), per k-block: S^T = kpad_ik @ q^T (runs), P~^T =
    exp(S^T/8) (ScalarE, one op per (ik, 1024-window)), PV accumulated
    into pa with the ones-column denominator trick (row 64).
  - normalization: only the two PSUM->SBUF copies are eager; the
    reciprocal/broadcast/multiply chain (which round-trips SBUF DMAs)
    is deferred and spread over the next head's iterations so it never
    head-of-line-blocks the in-order Vector/GpSimd queues.
  - projection is a 4-deep PSUM pipeline (alternating ring slots) with
    PSUM->SBUF copies alternating Vector/Scalar and per-tile output
    DMAs; m-tiles ordered so the last head's deferred normalize chain
    overlaps the first half of proj.
"""

import numpy as np
from ml_dtypes import bfloat16

import concourse.mybir as mybir
import concourse.tile as tile
from concourse import bacc
from concourse.bass_utils import run_bass_kernel_spmd

B, S, D, H = 2, 2048, 1024, 16
HD = 64          # head dim
BS = 128         # mask block size
NB = S // BS     # 16 blocks per axis
HPC = 4          # heads per core
N_CORES = 8
SCALE = HD ** -0.5
KT = D // 128    # 8 k-tiles over the embedding dim
VW = HPC * (HD + 1)  # 260

F32 = mybir.dt.float32
F32R = mybir.dt.float32r
BF16 = mybir.dt.bfloat16
EXP = mybir.ActivationFunctionType.Exp

_program_cache: dict[bytes, object] = {}


def _plan_runs_g(vis, last_vis, ik, g):
    """Contiguous visible q-block runs for k-block ik within 1024-col
    window g. Runs break at 4-block (512-col = PSUM bank) boundaries."""
    runs = []
    jq, end = 8 * g, 8 * g + 8
    while jq < end:
        if not vis[jq][ik]:
            jq += 1
            continue
        start = jq
        while jq + 1 < end and vis[jq + 1][ik] and (jq + 1) % 4 != 0:
            jq += 1
        stopf = any(last_vis[b] == ik for b in range(start, jq + 1))
        runs.append((start, jq - start + 1, stopf))
        jq += 1
    return runs


def _build_program(mask: np.ndarray):
    vis = [[ik <= jq or not bool(mask[jq, ik]) for ik in range(NB)]
           for jq in range(NB)]
    last_vis = [max(ik for ik in range(NB) if vis[jq][ik]) for jq in range(NB)]
    lastw = [max(last_vis[w * 4:(w + 1) * 4]) for w in range(4)]
    RUNS = {(g, ik): _plan_runs_g(vis, last_vis, ik, g)
            for g in range(2) for ik in range(NB)}

    nc = bacc.Bacc("TRN2", target_bir_lowering=False, debug=False,
                   num_devices=N_CORES)
    # host pre-packs everything 128-partition-major and fully contiguous:
    # xT_sl: [128, 4*8*512]  slice-major: slice s (512 seq cols), then k-tile
    # wqk_sl: [128, 8*512]   k-tile major; within: [q0|q1][k0|k1][q2|q3][k2|k3]
    # wv_sl:  [128, 8*260]   k-tile major
    xT_d = nc.dram_tensor("xT", [128, 4 * KT * 512], BF16, kind="ExternalInput")
    wqk_d = nc.dram_tensor("wqk", [128, KT * 512], BF16, kind="ExternalInput")
    wv_d = nc.dram_tensor("wv", [128, KT * VW], BF16, kind="ExternalInput")
    wpr_d = nc.dram_tensor("wpr", [HPC * HD, D], F32R, kind="ExternalInput")
    out_d = nc.dram_tensor("out", [S, D], BF16, kind="ExternalOutput")

    with tile.TileContext(nc) as tc:
        with tc.tile_pool(name="pp", bufs=1) as pp, \
             tc.tile_pool(name="ptp", bufs=5) as ptp, \
             tc.tile_pool(name="ps", bufs=2, space="PSUM") as ps:
            # ---- persistent SBUF tiles ----
            xT_sl = pp.tile([128, 4 * KT * 512], BF16, tag="xT", name="xT")
            wqk_sl = pp.tile([128, KT * 512], BF16, tag="wqk", name="wqk")
            wv_sl = pp.tile([128, KT * VW], BF16, tag="wv", name="wv")
            wpr_t = [pp.tile([128, D], F32R, tag=f"wpr{k}", name=f"wpr{k}")
                     for k in range(2)]
            q_t = [pp.tile([128, S], F32R, tag=f"q{p}", name=f"q{p}")
                   for p in range(2)]
            kpad_t = [pp.tile([128, S], F32R, tag=f"kp{h}", name=f"kp{h}")
                      for h in range(HPC)]
            v_t = [pp.tile([128, VW], F32R, tag=f"v{m}", name=f"v{m}")
                   for m in range(NB)]
            attn_t = [pp.tile([128, S], F32R, tag=f"attn{i}", name=f"attn{i}")
                      for i in range(2)]
            d16_t = pp.tile([128, 8 * HPC * 2], F32, tag="d16", name="d16")
            r0_t = pp.tile([1, S], F32, tag="r0", name="r0")
            onec = pp.tile([128, 4], F32, tag="onec", name="onec")
            zsrc = pp.tile([64, 512], F32, tag="zsrc", name="zsrc")
            scr = pp.tile([128, 4], F32, tag="scr", name="scr")

            # ---- init + ACT table pre-warm ----
            nc.vector.memset(onec[:], 1.0)
            nc.vector.memset(zsrc[:], 0.0)
            nc.scalar.activation(scr[:], onec[:], EXP, scale=1.0)
            for h in range(HPC):
                z0 = 64 if h % 2 == 0 else 0
                for c in range(4):
                    eng = nc.vector if (h * 4 + c) % 2 == 0 else nc.scalar
                    cs = c * 512
                    if eng is nc.vector:
                        eng.tensor_copy(kpad_t[h][z0:z0 + 64, cs:cs + 512],
                                        zsrc[:])
                    else:
                        eng.copy(kpad_t[h][z0:z0 + 64, cs:cs + 512], zsrc[:])

            # ---- input DMAs: few big contiguous pieces, consumption order,
            # issue alternating between the sync and gpsimd queues (descriptor
            # generation is ~0.6us each and serial per queue) ----
            dmact = [0]

            def in_dma(dst, src):
                eng = nc.sync if dmact[0] % 2 == 0 else nc.gpsimd
                dmact[0] += 1
                eng.dma_start(out=dst, in_=src)

            for h in range(4):  # wqk + xT s0 interleaved: 8 x 256KB
                o = h * 1024
                in_dma(wqk_sl[:, o:o + 1024], wqk_d[:, o:o + 1024])
                in_dma(xT_sl[:, o:o + 1024], xT_d[:, o:o + 1024])
            for h in range(2):  # xT s1: 2 x 512KB
                o = 4096 + h * 2048
                in_dma(xT_sl[:, o:o + 2048], xT_d[:, o:o + 2048])
            hw = KT * VW // 2
            for h in range(2):  # wv: 2 x 265KB
                in_dma(wv_sl[:, h * hw:(h + 1) * hw],
                       wv_d[:, h * hw:(h + 1) * hw])
            for p4 in range(4):  # xT s2+s3: 4 x 512KB
                o = 2 * 4096 + p4 * 2048
                in_dma(xT_sl[:, o:o + 2048], xT_d[:, o:o + 2048])
            for k in range(2):
                in_dma(wpr_t[k][:], wpr_d[k * 128:(k + 1) * 128, :])

            # ---- gen chunk emitters (copies alternate Vector/Scalar) ----
            genct = [0]

            def qk_chunk(p, t, c, lead=False):
                """[128,512] chunk of q-pair (t=0) or k-pair (t=1) tile."""
                pb = ps.tile([128, 512], F32, tag="st", bufs=3, name=f"pb{p}{t}{c}")
                off = p * 256 + t * 128
                cs = c * 512
                for k in range(KT):
                    nc.tensor.matmul(
                        pb[:], wqk_sl[:, k * 512 + off:k * 512 + off + 128],
                        xT_sl[:, c * 4096 + k * 512:c * 4096 + (k + 1) * 512],
                        start=(k == 0), stop=(k == KT - 1))
                genct[0] += 1
                use_sc = genct[0] % 2 == 1
                # keep both half-copies of one chunk on ONE engine: the
                # framework serializes sibling readers cross-engine, which
                # couples the exp stream to the Vector queue otherwise
                if t == 0:
                    if use_sc:
                        nc.scalar.copy(q_t[p][:, cs:cs + 512], pb[:])
                    else:
                        nc.vector.tensor_copy(q_t[p][:, cs:cs + 512], pb[:])
                else:
                    h0, h1 = 2 * p, 2 * p + 1
                    if use_sc:
                        nc.scalar.copy(kpad_t[h0][0:64, cs:cs + 512],
                                       pb[0:64, :])
                        nc.scalar.copy(kpad_t[h1][64:128, cs:cs + 512],
                                       pb[64:128, :])
                    else:
                        nc.vector.tensor_copy(kpad_t[h0][0:64, cs:cs + 512],
                                              pb[0:64, :])
                        nc.vector.tensor_copy(kpad_t[h1][64:128, cs:cs + 512],
                                              pb[64:128, :])

            def v_chunk(m):
                pc = ps.tile([128, 512], F32, tag="st", bufs=3, name=f"pc{m}")
                s, r = m // 4, m % 4
                for k in range(KT):
                    nc.tensor.matmul(
                        pc[:, 0:VW],
                        xT_sl[:, s * 4096 + k * 512 + r * 128:
                               s * 4096 + k * 512 + (r + 1) * 128],
                        wv_sl[:, k * VW:(k + 1) * VW],
                        start=(k == 0), stop=(k == KT - 1))
                nc.vector.tensor_copy(v_t[m][:], pc[:, 0:VW])
                nc.vector.tensor_copy(v_t[m][:, HD::HD + 1], onec[:])

            # ---- deferred-op machinery ----
            deferred = []  # [countdown, fn]

            def poll_deferred():
                due = [d for d in deferred if d[0] <= 1]
                for d in due:
                    deferred.remove(d)
                for d in deferred:
                    d[0] -= 1
                for d in due:
                    d[1]()

            def force_deferred(keep=None):
                kept = []
                while deferred:
                    d = deferred.pop(0)
                    if keep is not None and d[2] == keep:
                        kept.append(d)
                    else:
                        d[1]()
                deferred.extend(kept)

            # ---- normalize chain ----
            first_mm = {}   # (j, w) -> True once consumed
            wins_done = {}  # (j, g) -> count

            # per-head staging, ring-allocated (lifetimes span into next head).
            # stage[0:64] = unnormalized attn rows, stage[64:65] = denominator.
            cur = {"stage": None, "odd": None}

            def enqueue_chain(j, g, spacing):
                p, gc = j // 2, g * 1024
                sl = d16_t[:, (2 * j + g) * 8:(2 * j + g + 1) * 8]
                stage, odd = cur["stage"], cur["odd"]
                if j % 2 == 0:
                    dst = attn_t[p][0:64, gc:gc + 1024]
                else:
                    dst = odd[0:64, gc:gc + 1024]

                def s1():
                    nc.gpsimd.dma_start(out=sl, in_=stage[64:65, gc:gc + 1024])

                def s2():
                    nc.vector.reciprocal(sl, sl)

                def s3():
                    nc.gpsimd.dma_start(out=r0_t[0:1, gc:gc + 1024], in_=sl)

                def s4(h):
                    hc = gc + h * 512
                    dbc = pp.tile([64, 512], F32, tag="dbc", bufs=4,
                                  name=f"dbc{j}{g}{h}")
                    cur[f"dbc{j}{g}{h}"] = dbc
                    nc.gpsimd.partition_broadcast(dbc[:],
                                                  r0_t[0:1, hc:hc + 512])

                def s5(h):
                    hc = gc + h * 512
                    dbc = cur.pop(f"dbc{j}{g}{h}")
                    nc.vector.tensor_mul(dst[:, h * 512:(h + 1) * 512],
                                         stage[0:64, hc:hc + 512], dbc[:])

                def s6():
                    nc.gpsimd.dma_start(out=attn_t[p][64:128, gc:gc + 1024],
                                        in_=odd[0:64, gc:gc + 1024])

                # both PBs issued before the MULs so the GpSimd latency is
                # hidden before the Vector ops need the result
                steps = [(1, s1), (2, s2), (1, s3),
                         (1, lambda: s4(0)), (1, lambda: s4(1)),
                         (2, lambda: s5(0)), (1, lambda: s5(1))]
                if j % 2 == 1:
                    steps.append((1, s6))
                cd = 0
                for extra, fn in steps:
                    cd += spacing * extra
                    deferred.append([cd, fn, (j, g)])

            def norm_copies(j, g, ik, pa_g):
                for w in (2 * g, 2 * g + 1):
                    if lastw[w] != ik:
                        continue
                    ws = w * 512
                    rel = ws - g * 1024
                    nc.vector.tensor_copy(cur["stage"][0:65, ws:ws + 512],
                                          pa_g[0:65, rel:rel + 512])
                    wins_done[(j, g)] = wins_done.get((j, g), 0) + 1
                    if wins_done[(j, g)] == 2:
                        enqueue_chain(j, g, spacing=(1 if j == 2 else 2))

            # ---- attention pipeline ----
            pending = [None]  # [(j, g, ik, runs, ptg, pa_g)]

            def flush_pending():
                item = pending[0]
                pending[0] = None
                if item is None:
                    return
                j, g, ik, runs, ptg, pa_g = item
                lhsT_v = v_t[ik][:, j * (HD + 1):(j + 1) * (HD + 1)]
                for (qb0, nbk, stopf) in runs:
                    qs, qlen = qb0 * 128, nbk * 128
                    rel = qs - g * 1024
                    w = qb0 // 4
                    startf = first_mm.pop((j, w), False)
                    nc.tensor.matmul(pa_g[0:65, rel:rel + qlen], lhsT_v,
                                     ptg[:, rel:rel + qlen],
                                     start=startf, stop=stopf,
                                     skip_group_check=True)
                norm_copies(j, g, ik, pa_g)

            def attn_iter(j, g, ik, pa_g, gen=None):
                runs = RUNS[(g, ik)]
                stg = ps.tile([128, 1024], F32, tag="st", bufs=3, name=f"st{j}{g}{ik}")
                lhsT_k = kpad_t[j][:, ik * 128:(ik + 1) * 128]
                qtile = q_t[j // 2]
                for (qb0, nbk, stopf) in runs:
                    qs, qlen = qb0 * 128, nbk * 128
                    rel = qs - g * 1024
                    nc.tensor.matmul(stg[:, rel:rel + qlen], lhsT_k,
                                     qtile[:, qs:qs + qlen],
                                     start=True, stop=True)
                lo = min(r[0] for r in runs) * 128 - g * 1024
                hi = (max(r[0] + r[1] for r in runs)) * 128 - g * 1024
                ptg = ptp.tile([128, 1024], F32R, tag="pt", name=f"pt{j}{g}{ik}")
                nc.scalar.activation(ptg[:, lo:hi], stg[:, lo:hi], EXP,
                                     scale=SCALE)
                if gen is not None:
                    gen()
                poll_deferred()
                flush_pending()
                pending[0] = (j, g, ik, runs, ptg, pa_g)

            # ---- lead: qk-gen for pair 0, windows g=0 ----
            for (t, c) in ((0, 0), (1, 0), (0, 1), (1, 1)):
                qk_chunk(0, t, c, lead=True)

            iters = {g: [ik for ik in range(NB) if RUNS[(g, ik)]]
                     for g in range(2)}

            def head_items(j):
                # g=0 fully then g=1: only ONE pa tile live at a time, which
                # frees 2 PSUM banks for a 3-deep stg ring (the exp pipeline)
                phases = [(0, iters[0]), (1, iters[1])]
                return [(g, ik) for (g, iklist) in phases for ik in iklist]

            def edf_schedule(items, chunks):
                """Assign gen chunks to iteration slots by earliest deadline.
                chunks: list of (deadline_slot_inclusive, fn). Returns
                slot -> [fns]; infeasible chunks go to slot 0."""
                slots = {i: [] for i in range(len(items))}
                fill = {i: 0 for i in range(len(items))}
                for dl, fn in sorted(chunks, key=lambda c: c[0]):
                    placed = False
                    # latest-fit: emit just-in-time so gen MMs queue behind
                    # already-arrived DMA data instead of stalling the PE
                    for s in range(min(dl, len(items) - 1), -1, -1):
                        if fill[s] < 2:
                            slots[s].append(fn)
                            fill[s] += 1
                            placed = True
                            break
                    if not placed:
                        slots[0].insert(0, fn)
                return slots

            def head1_chunks(items):
                """v tiles (PV deadline) + pair-0 windows g=1 (QK deadline)."""
                chunks = []
                for m in range(NB):
                    idx = min((i for i, (g, ik) in enumerate(items) if ik == m),
                              default=0)
                    chunks.append((idx + 1, lambda m=m: v_chunk(m)))
                for c in (2, 3):
                    # q chunk c: first QK of window-pair g=c//2 touching it
                    idx = min((i for i, (g, ik) in enumerate(items)
                               if g == c // 2), default=1)
                    chunks.append((max(0, idx - 1),
                                   lambda c=c: qk_chunk(0, 0, c)))
                    # kpad chunk c: first QK with ik in [4c, 4c+4)
                    idx = min((i for i, (g, ik) in enumerate(items)
                               if 4 * c <= ik < 4 * c + 4), default=1)
                    chunks.append((max(0, idx - 1),
                                   lambda c=c: qk_chunk(0, 1, c)))
                return chunks

            def head0_chunks(items):
                """pair-1 gen, needed only by heads 3/2: spread evenly."""
                chunks = []
                pos = 0
                for c in range(4):
                    for t in (0, 1):
                        chunks.append((pos, lambda t=t, c=c: qk_chunk(1, t, c)))
                        pos += 3
                return chunks

            # ---- projection tile emitter (used by proj loop AND as PE
            # filler in head 2's g=1 pass, once g0 windows are final) ----
            projct = [0]

            def emit_proj(m):
                i = projct[0]
                projct[0] += 1
                po = ps.tile([128, D], F32, tag="st", bufs=3, name=f"po{m}")
                for kt in range(2):
                    for c in range(2):
                        nc.tensor.matmul(
                            po[:, c * 512:(c + 1) * 512],
                            attn_t[kt][:, m * 128:(m + 1) * 128],
                            wpr_t[kt][:, c * 512:(c + 1) * 512],
                            start=(kt == 0), stop=(kt == 1))
                ob = pp.tile([128, D], BF16, tag="ob", bufs=3, name=f"ob{m}")
                if i % 2 == 0:
                    nc.vector.tensor_copy(ob[:], po[:])
                else:
                    nc.scalar.copy(ob[:], po[:])
                if m >= 12:
                    # tail tiles: split across both queue engines so the
                    # final drain is half a tile, not a whole one
                    nc.sync.dma_start(out=out_d[m * 128:m * 128 + 64, :],
                                      in_=ob[0:64, :])
                    nc.gpsimd.dma_start(out=out_d[m * 128 + 64:(m + 1) * 128, :],
                                        in_=ob[64:128, :])
                else:
                    deng = nc.sync if i % 2 == 0 else nc.gpsimd
                    deng.dma_start(out=out_d[m * 128:(m + 1) * 128, :],
                                   in_=ob[:])

            for j in (1, 0, 3, 2):
                items = head_items(j)
                if j == 1:
                    genmap = edf_schedule(items, head1_chunks(items))
                elif j == 0:
                    genmap = edf_schedule(items, head0_chunks(items))
                else:
                    genmap = {}
                for w in range(4):
                    first_mm[(j, w)] = True
                cur["stage"] = pp.tile([65, S], F32, tag="stage", bufs=2,
                                       name=f"stage{j}")
                if j % 2 == 1:
                    cur["odd"] = pp.tile([64, S], F32R, tag="odd", bufs=1,
                                         name=f"odd{j}")
                pa = {}
                for i, (g, ik) in enumerate(items):
                    if g not in pa:
                        pa[g] = ps.tile([65, 1024], F32, tag="pa", bufs=1,
                                        name=f"pa{j}{g}")
                    fns = genmap.get(i, [])
                    gen = (lambda fns=fns: [f() for f in fns]) if fns else None
                    attn_iter(j, g, ik, pa[g], gen=gen)
                flush_pending()
                for w in range(4):
                    first_mm.pop((j, w), None)

            # ---- projection + output ----
            # flush all chains except the last head's g=1 (interleaved below)
            force_deferred(keep=(2, 1))
            last_chain = [d for d in deferred if d[2] == (2, 1)]
            deferred.clear()
            for m in range(16):
                if last_chain:
                    last_chain.pop(0)[1]()
                if m == 8:
                    while last_chain:
                        last_chain.pop(0)[1]()
                emit_proj(m)
            while last_chain:
                last_chain.pop(0)[1]()

    # consume first_mm flags at first-visible ik
    nc.compile()
    return nc


def _host_prep(x, w_qkv, w_proj):
    """Per-core input slices, packed 128-partition-major and contiguous.
    x/wqk/wv in bf16, wpr in f32."""
    # xT_sl[b]: [128, 4*8*512] slice-major then k-tile-major
    xT_sl = []
    for b in range(B):
        xT = x[b].T.astype(bfloat16)  # [D, S]
        arr = np.empty((128, 4 * KT * 512), bfloat16)
        for s in range(4):
            for k in range(KT):
                arr[:, s * 4096 + k * 512:s * 4096 + (k + 1) * 512] = \
                    xT[k * 128:(k + 1) * 128, s * 512:(s + 1) * 512]
        xT_sl.append(np.ascontiguousarray(arr))
    in_maps = []
    for c in range(N_CORES):
        b, grp = c // 4, c % 4
        heads = list(range(grp * HPC, (grp + 1) * HPC))
        wqk = np.empty((D, 2 * HPC * HD), np.float32)
        wv = np.zeros((D, VW), np.float32)
        wpr = np.empty((HPC * HD, D), np.float32)
        for j, h in enumerate(heads):
            p, i = j // 2, j % 2  # pair, index in pair
            # pair block: [q_a|q_b][k_a|k_b] at 256*p
            wqk[:, p * 256 + i * HD:p * 256 + (i + 1) * HD] = \
                w_qkv[:, h * HD:(h + 1) * HD]
            wqk[:, p * 256 + 128 + i * HD:p * 256 + 128 + (i + 1) * HD] = \
                w_qkv[:, D + h * HD:D + (h + 1) * HD]
            wv[:, j * (HD + 1):j * (HD + 1) + HD] = \
                w_qkv[:, 2 * D + h * HD:2 * D + (h + 1) * HD]
            wpr[j * HD:(j + 1) * HD, :] = w_proj[h * HD:(h + 1) * HD, :]
        # repack k-tile-major [128, KT*cols]
        wqk_sl = np.empty((128, KT * 512), bfloat16)
        wv_sl = np.empty((128, KT * VW), bfloat16)
        for k in range(KT):
            wqk_sl[:, k * 512:(k + 1) * 512] = \
                wqk[k * 128:(k + 1) * 128, :].astype(bfloat16)
            wv_sl[:, k * VW:(k + 1) * VW] = \
                wv[k * 128:(k + 1) * 128, :].astype(bfloat16)
        in_maps.append({
            "xT": xT_sl[b],
            "wqk": np.ascontiguousarray(wqk_sl),
            "wv": np.ascontiguousarray(wv_sl),
            "wpr": np.ascontiguousarray(wpr),
        })
    return in_maps


def get_program(block_mask: np.ndarray):
    key = np.asarray(block_mask, bool).tobytes()
    if key not in _program_cache:
        _program_cache[key] = _build_program(np.asarray(block_mask, bool))
    return _program_cache[key]


def kernel(x, w_qkv, w_proj, b_proj, block_mask):
    x = np.asarray(x, np.float32)
    w_qkv = np.asarray(w_qkv, np.float32)
    w_proj = np.asarray(w_proj, np.float32)
    b_proj = np.asarray(b_proj, np.float32)
    nc = get_program(block_mask)
    in_maps = _host_prep(x, w_qkv, w_proj)
    res = run_bass_kernel_spmd(nc, in_maps, core_ids=list(range(N_CORES)))
    out = np.empty((B, S, D), np.float32)
    for b in range(B):
        acc = np.asarray(res.results[4 * b]["out"], np.float64)
        for g in range(1, 4):
            acc = acc + np.asarray(res.results[4 * b + g]["out"], np.float64)
        out[b] = (acc + b_proj).astype(np.float32)
    return out


# revision 54
# speedup vs baseline: 1.0031x; 1.0031x over previous
"""Block-sparse multi-head attention on 8 Trainium2 NeuronCores.

Problem: y = proj(softmax(mask(q @ k^T / sqrt(hd))) @ v) for
B=2, S=2048, D=1024, H=16 heads, block size 128, with a [16,16] boolean
block mask (True = masked) applied to strictly-upper (k-block > q-block)
blocks.

Sharding: batch x head-group. Core c handles batch c//4 and heads
[4*(c%4), 4*(c%4)+4). No collectives: the host pre-slices inputs
(including pre-transposing x to x^T) and sums the 4 per-batch partial
projection outputs on the way out.

This version fuses all phases into one software-pipelined instruction
stream to keep ScalarE (the exp bottleneck, ~100us/core) and the PE
(~123us/core) simultaneously busy:
  - x/w_qkv/w_v are uploaded in bf16 (halves input DMA to ~6.5MB);
    DMAs are chunked by xT column-slice and issued in consumption order
    so the first attention exp lands ~8us into the kernel.
  - qk-gen for head pair 0 runs first; v-gen and pair-1 qk-gen chunks
    are interleaved into the attention pipeline of heads 1 and 0
    (sharing one PSUM ring) so the PE never idles long enough for HAM
    to re-throttle the clock.
  - attention per head runs as two window passes g=0/1 (pa [65,1024]
    PSUM x2-ring), per k-block: S^T = kpad_ik @ q^T (runs), P~^T =
    exp(S^T/8) (ScalarE, one op per (ik, 1024-window)), PV accumulated
    into pa with the ones-column denominator trick (row 64).
  - normalization: only the two PSUM->SBUF copies are eager; the
    reciprocal/broadcast/multiply chain (which round-trips SBUF DMAs)
    is deferred and spread over the next head's iterations so it never
    head-of-line-blocks the in-order Vector/GpSimd queues.
  - projection is a 4-deep PSUM pipeline (alternating ring slots) with
    PSUM->SBUF copies alternating Vector/Scalar and per-tile output
    DMAs; m-tiles ordered so the last head's deferred normalize chain
    overlaps the first half of proj.
"""

import numpy as np
from ml_dtypes import bfloat16

import concourse.mybir as mybir
import concourse.tile as tile
from concourse import bacc
from concourse.bass_utils import run_bass_kernel_spmd

B, S, D, H = 2, 2048, 1024, 16
HD = 64          # head dim
BS = 128         # mask block size
NB = S // BS     # 16 blocks per axis
HPC = 4          # heads per core
N_CORES = 8
SCALE = HD ** -0.5
KT = D // 128    # 8 k-tiles over the embedding dim
VW = HPC * (HD + 1)  # 260

F32 = mybir.dt.float32
F32R = mybir.dt.float32r
BF16 = mybir.dt.bfloat16
EXP = mybir.ActivationFunctionType.Exp

_program_cache: dict[bytes, object] = {}


def _plan_runs_g(vis, last_vis, ik, g):
    """Contiguous visible q-block runs for k-block ik within 1024-col
    window g. Runs break at 4-block (512-col = PSUM bank) boundaries."""
    runs = []
    jq, end = 8 * g, 8 * g + 8
    while jq < end:
        if not vis[jq][ik]:
            jq += 1
            continue
        start = jq
        while jq + 1 < end and vis[jq + 1][ik] and (jq + 1) % 4 != 0:
            jq += 1
        stopf = any(last_vis[b] == ik for b in range(start, jq + 1))
        runs.append((start, jq - start + 1, stopf))
        jq += 1
    return runs


def _build_program(mask: np.ndarray):
    vis = [[ik <= jq or not bool(mask[jq, ik]) for ik in range(NB)]
           for jq in range(NB)]
    last_vis = [max(ik for ik in range(NB) if vis[jq][ik]) for jq in range(NB)]
    lastw = [max(last_vis[w * 4:(w + 1) * 4]) for w in range(4)]
    RUNS = {(g, ik): _plan_runs_g(vis, last_vis, ik, g)
            for g in range(2) for ik in range(NB)}

    nc = bacc.Bacc("TRN2", target_bir_lowering=False, debug=False,
                   num_devices=N_CORES)
    # host pre-packs everything 128-partition-major and fully contiguous:
    # xT_sl: [128, 4*8*512]  slice-major: slice s (512 seq cols), then k-tile
    # wqk_sl: [128, 8*512]   k-tile major; within: [q0|q1][k0|k1][q2|q3][k2|k3]
    # wv_sl:  [128, 8*260]   k-tile major
    xT_d = nc.dram_tensor("xT", [128, 4 * KT * 512], BF16, kind="ExternalInput")
    wqk_d = nc.dram_tensor("wqk", [128, KT * 512], BF16, kind="ExternalInput")
    wv_d = nc.dram_tensor("wv", [128, KT * VW], BF16, kind="ExternalInput")
    wpr_d = nc.dram_tensor("wpr", [HPC * HD, D], F32R, kind="ExternalInput")
    out_d = nc.dram_tensor("out", [S, D], BF16, kind="ExternalOutput")

    with tile.TileContext(nc) as tc:
        with tc.tile_pool(name="pp", bufs=1) as pp, \
             tc.tile_pool(name="ptp", bufs=5) as ptp, \
             tc.tile_pool(name="ps", bufs=2, space="PSUM") as ps:
            # ---- persistent SBUF tiles ----
            xT_sl = pp.tile([128, 4 * KT * 512], BF16, tag="xT", name="xT")
            wqk_sl = pp.tile([128, KT * 512], BF16, tag="wqk", name="wqk")
            wv_sl = pp.tile([128, KT * VW], BF16, tag="wv", name="wv")
            wpr_t = [pp.tile([128, D], F32R, tag=f"wpr{k}", name=f"wpr{k}")
                     for k in range(2)]
            q_t = [pp.tile([128, S], F32R, tag=f"q{p}", name=f"q{p}")
                   for p in range(2)]
            kpad_t = [pp.tile([128, S], F32R, tag=f"kp{h}", name=f"kp{h}")
                      for h in range(HPC)]
            v_t = [pp.tile([128, VW], F32R, tag=f"v{m}", name=f"v{m}")
                   for m in range(NB)]
            attn_t = [pp.tile([128, S], F32R, tag=f"attn{i}", name=f"attn{i}")
                      for i in range(2)]
            d16_t = pp.tile([128, 8 * HPC * 2], F32, tag="d16", name="d16")
            r0_t = pp.tile([1, S], F32, tag="r0", name="r0")
            onec = pp.tile([128, 4], F32, tag="onec", name="onec")
            zsrc = pp.tile([64, 512], F32, tag="zsrc", name="zsrc")
            scr = pp.tile([128, 4], F32, tag="scr", name="scr")

            # ---- init + ACT table pre-warm ----
            nc.vector.memset(onec[:], 1.0)
            nc.vector.memset(zsrc[:], 0.0)
            nc.scalar.activation(scr[:], onec[:], EXP, scale=1.0)
            for h in range(HPC):
                z0 = 64 if h % 2 == 0 else 0
                for c in range(4):
                    eng = nc.vector if (h * 4 + c) % 2 == 0 else nc.scalar
                    cs = c * 512
                    if eng is nc.vector:
                        eng.tensor_copy(kpad_t[h][z0:z0 + 64, cs:cs + 512],
                                        zsrc[:])
                    else:
                        eng.copy(kpad_t[h][z0:z0 + 64, cs:cs + 512], zsrc[:])

            # ---- input DMAs: few big contiguous pieces, consumption order,
            # issue alternating between the sync and gpsimd queues (descriptor
            # generation is ~0.6us each and serial per queue) ----
            dmact = [0]

            def in_dma(dst, src):
                eng = nc.sync if dmact[0] % 2 == 0 else nc.gpsimd
                dmact[0] += 1
                eng.dma_start(out=dst, in_=src)

            for h in range(4):  # wqk + xT s0 interleaved: 8 x 256KB
                o = h * 1024
                in_dma(wqk_sl[:, o:o + 1024], wqk_d[:, o:o + 1024])
                in_dma(xT_sl[:, o:o + 1024], xT_d[:, o:o + 1024])
            for h in range(2):  # xT s1: 2 x 512KB
                o = 4096 + h * 2048
                in_dma(xT_sl[:, o:o + 2048], xT_d[:, o:o + 2048])
            hw = KT * VW // 2
            for h in range(2):  # wv: 2 x 265KB
                in_dma(wv_sl[:, h * hw:(h + 1) * hw],
                       wv_d[:, h * hw:(h + 1) * hw])
            for p4 in range(4):  # xT s2+s3: 4 x 512KB
                o = 2 * 4096 + p4 * 2048
                in_dma(xT_sl[:, o:o + 2048], xT_d[:, o:o + 2048])
            for k in range(2):
                in_dma(wpr_t[k][:], wpr_d[k * 128:(k + 1) * 128, :])

            # ---- gen chunk emitters (copies alternate Vector/Scalar) ----
            genct = [0]

            def qk_chunk(p, t, c, lead=False):
                """[128,512] chunk of q-pair (t=0) or k-pair (t=1) tile."""
                pb = ps.tile([128, 512], F32, tag="st", bufs=3, name=f"pb{p}{t}{c}")
                off = p * 256 + t * 128
                cs = c * 512
                for k in range(KT):
                    nc.tensor.matmul(
                        pb[:], wqk_sl[:, k * 512 + off:k * 512 + off + 128],
                        xT_sl[:, c * 4096 + k * 512:c * 4096 + (k + 1) * 512],
                        start=(k == 0), stop=(k == KT - 1))
                genct[0] += 1
                use_sc = genct[0] % 2 == 1
                # keep both half-copies of one chunk on ONE engine: the
                # framework serializes sibling readers cross-engine, which
                # couples the exp stream to the Vector queue otherwise
                if t == 0:
                    if use_sc:
                        nc.scalar.copy(q_t[p][:, cs:cs + 512], pb[:])
                    else:
                        nc.vector.tensor_copy(q_t[p][:, cs:cs + 512], pb[:])
                else:
                    h0, h1 = 2 * p, 2 * p + 1
                    if use_sc:
                        nc.scalar.copy(kpad_t[h0][0:64, cs:cs + 512],
                                       pb[0:64, :])
                        nc.scalar.copy(kpad_t[h1][64:128, cs:cs + 512],
                                       pb[64:128, :])
                    else:
                        nc.vector.tensor_copy(kpad_t[h0][0:64, cs:cs + 512],
                                              pb[0:64, :])
                        nc.vector.tensor_copy(kpad_t[h1][64:128, cs:cs + 512],
                                              pb[64:128, :])

            def v_chunk(m):
                pc = ps.tile([128, 512], F32, tag="st", bufs=3, name=f"pc{m}")
                s, r = m // 4, m % 4
                for k in range(KT):
                    nc.tensor.matmul(
                        pc[:, 0:VW],
                        xT_sl[:, s * 4096 + k * 512 + r * 128:
                               s * 4096 + k * 512 + (r + 1) * 128],
                        wv_sl[:, k * VW:(k + 1) * VW],
                        start=(k == 0), stop=(k == KT - 1))
                nc.vector.tensor_copy(v_t[m][:], pc[:, 0:VW])
                nc.vector.tensor_copy(v_t[m][:, HD::HD + 1], onec[:])

            # ---- deferred-op machinery ----
            deferred = []  # [countdown, fn]

            def poll_deferred():
                due = [d for d in deferred if d[0] <= 1]
                for d in due:
                    deferred.remove(d)
                for d in deferred:
                    d[0] -= 1
                for d in due:
                    d[1]()

            def force_deferred(keep=None):
                kept = []
                while deferred:
                    d = deferred.pop(0)
                    if keep is not None and d[2] == keep:
                        kept.append(d)
                    else:
                        d[1]()
                deferred.extend(kept)

            # ---- normalize chain ----
            first_mm = {}   # (j, w) -> True once consumed
            wins_done = {}  # (j, g) -> count

            # per-head staging, ring-allocated (lifetimes span into next head).
            # stage[0:64] = unnormalized attn rows, stage[64:65] = denominator.
            cur = {"stage": None, "odd": None}

            def enqueue_chain(j, g, spacing):
                p, gc = j // 2, g * 1024
                sl = d16_t[:, (2 * j + g) * 8:(2 * j + g + 1) * 8]
                stage, odd = cur["stage"], cur["odd"]
                if j % 2 == 0:
                    dst = attn_t[p][0:64, gc:gc + 1024]
                else:
                    dst = odd[0:64, gc:gc + 1024]

                def s1():
                    nc.gpsimd.dma_start(out=sl, in_=stage[64:65, gc:gc + 1024])

                def s2():
                    nc.vector.reciprocal(sl, sl)

                def s3():
                    nc.gpsimd.dma_start(out=r0_t[0:1, gc:gc + 1024], in_=sl)

                def s4(h):
                    hc = gc + h * 512
                    dbc = pp.tile([64, 512], F32, tag="dbc", bufs=4,
                                  name=f"dbc{j}{g}{h}")
                    cur[f"dbc{j}{g}{h}"] = dbc
                    nc.gpsimd.partition_broadcast(dbc[:],
                                                  r0_t[0:1, hc:hc + 512])

                def s5(h):
                    hc = gc + h * 512
                    dbc = cur.pop(f"dbc{j}{g}{h}")
                    nc.vector.tensor_mul(dst[:, h * 512:(h + 1) * 512],
                                         stage[0:64, hc:hc + 512], dbc[:])

                def s6():
                    nc.gpsimd.dma_start(out=attn_t[p][64:128, gc:gc + 1024],
                                        in_=odd[0:64, gc:gc + 1024])

                # both PBs issued before the MULs so the GpSimd latency is
                # hidden before the Vector ops need the result
                steps = [(1, s1), (2, s2), (1, s3),
                         (1, lambda: s4(0)), (1, lambda: s4(1)),
                         (2, lambda: s5(0)), (1, lambda: s5(1))]
                if j % 2 == 1:
                    steps.append((1, s6))
                cd = 0
                for extra, fn in steps:
                    cd += spacing * extra
                    deferred.append([cd, fn, (j, g)])

            def norm_copies(j, g, ik, pa_g):
                for w in (2 * g, 2 * g + 1):
                    if lastw[w] != ik:
                        continue
                    ws = w * 512
                    rel = ws - g * 1024
                    nc.vector.tensor_copy(cur["stage"][0:65, ws:ws + 512],
                                          pa_g[0:65, rel:rel + 512])
                    wins_done[(j, g)] = wins_done.get((j, g), 0) + 1
                    if wins_done[(j, g)] == 2:
                        enqueue_chain(j, g, spacing=(1 if j == 2 else 2))

            # ---- attention pipeline ----
            pending = [None]  # [(j, g, ik, runs, ptg, pa_g)]

            def flush_pending():
                item = pending[0]
                pending[0] = None
                if item is None:
                    return
                j, g, ik, runs, ptg, pa_g = item
                lhsT_v = v_t[ik][:, j * (HD + 1):(j + 1) * (HD + 1)]
                for (qb0, nbk, stopf) in runs:
                    qs, qlen = qb0 * 128, nbk * 128
                    rel = qs - g * 1024
                    w = qb0 // 4
                    startf = first_mm.pop((j, w), False)
                    nc.tensor.matmul(pa_g[0:65, rel:rel + qlen], lhsT_v,
                                     ptg[:, rel:rel + qlen],
                                     start=startf, stop=stopf,
                                     skip_group_check=True)
                norm_copies(j, g, ik, pa_g)

            def attn_iter(j, g, ik, pa_g, gen=None):
                runs = RUNS[(g, ik)]
                stg = ps.tile([128, 1024], F32, tag="st", bufs=3, name=f"st{j}{g}{ik}")
                lhsT_k = kpad_t[j][:, ik * 128:(ik + 1) * 128]
                qtile = q_t[j // 2]
                for (qb0, nbk, stopf) in runs:
                    qs, qlen = qb0 * 128, nbk * 128
                    rel = qs - g * 1024
                    nc.tensor.matmul(stg[:, rel:rel + qlen], lhsT_k,
                                     qtile[:, qs:qs + qlen],
                                     start=True, stop=True)
                # one exp op per cluster of runs; split where the masked gap
                # exceeds 512 cols (gap cols cost more than a second op)
                ptg = ptp.tile([128, 1024], F32R, tag="pt", name=f"pt{j}{g}{ik}")
                clusters = []
                for (qb0, nbk, _s) in runs:
                    rlo = qb0 * 128 - g * 1024
                    rhi = rlo + nbk * 128
                    if clusters and rlo - clusters[-1][1] <= 512:
                        clusters[-1][1] = rhi
                    else:
                        clusters.append([rlo, rhi])
                for (clo, chi) in clusters:
                    nc.scalar.activation(ptg[:, clo:chi], stg[:, clo:chi], EXP,
                                         scale=SCALE)
                if gen is not None:
                    gen()
                poll_deferred()
                flush_pending()
                pending[0] = (j, g, ik, runs, ptg, pa_g)

            # ---- lead: qk-gen for pair 0, windows g=0 ----
            for (t, c) in ((0, 0), (1, 0), (0, 1), (1, 1)):
                qk_chunk(0, t, c, lead=True)

            iters = {g: [ik for ik in range(NB) if RUNS[(g, ik)]]
                     for g in range(2)}

            def head_items(j):
                # g=0 fully then g=1: only ONE pa tile live at a time, which
                # frees 2 PSUM banks for a 3-deep stg ring (the exp pipeline)
                phases = [(0, iters[0]), (1, iters[1])]
                return [(g, ik) for (g, iklist) in phases for ik in iklist]

            def edf_schedule(items, chunks):
                """Assign gen chunks to iteration slots by earliest deadline.
                chunks: list of (deadline_slot_inclusive, fn). Returns
                slot -> [fns]; infeasible chunks go to slot 0."""
                slots = {i: [] for i in range(len(items))}
                fill = {i: 0 for i in range(len(items))}
                for dl, fn in sorted(chunks, key=lambda c: c[0]):
                    placed = False
                    # latest-fit: emit just-in-time so gen MMs queue behind
                    # already-arrived DMA data instead of stalling the PE
                    for s in range(min(dl, len(items) - 1), -1, -1):
                        if fill[s] < 2:
                            slots[s].append(fn)
                            fill[s] += 1
                            placed = True
                            break
                    if not placed:
                        slots[0].insert(0, fn)
                return slots

            def head1_chunks(items):
                """v tiles (PV deadline) + pair-0 windows g=1 (QK deadline)."""
                chunks = []
                for m in range(NB):
                    idx = min((i for i, (g, ik) in enumerate(items) if ik == m),
                              default=0)
                    chunks.append((idx + 1, lambda m=m: v_chunk(m)))
                for c in (2, 3):
                    # q chunk c: first QK of window-pair g=c//2 touching it
                    idx = min((i for i, (g, ik) in enumerate(items)
                               if g == c // 2), default=1)
                    chunks.append((max(0, idx - 1),
                                   lambda c=c: qk_chunk(0, 0, c)))
                    # kpad chunk c: first QK with ik in [4c, 4c+4)
                    idx = min((i for i, (g, ik) in enumerate(items)
                               if 4 * c <= ik < 4 * c + 4), default=1)
                    chunks.append((max(0, idx - 1),
                                   lambda c=c: qk_chunk(0, 1, c)))
                return chunks

            def head0_chunks(items):
                """pair-1 gen, needed only by heads 3/2: spread evenly."""
                chunks = []
                pos = 0
                for c in range(4):
                    for t in (0, 1):
                        chunks.append((pos, lambda t=t, c=c: qk_chunk(1, t, c)))
                        pos += 3
                return chunks

            # ---- projection tile emitter (used by proj loop AND as PE
            # filler in head 2's g=1 pass, once g0 windows are final) ----
            projct = [0]

            def emit_proj(m):
                i = projct[0]
                projct[0] += 1
                po = ps.tile([128, D], F32, tag="st", bufs=3, name=f"po{m}")
                for kt in range(2):
                    for c in range(2):
                        nc.tensor.matmul(
                            po[:, c * 512:(c + 1) * 512],
                            attn_t[kt][:, m * 128:(m + 1) * 128],
                            wpr_t[kt][:, c * 512:(c + 1) * 512],
                            start=(kt == 0), stop=(kt == 1))
                ob = pp.tile([128, D], BF16, tag="ob", bufs=3, name=f"ob{m}")
                if i % 2 == 0:
                    nc.vector.tensor_copy(ob[:], po[:])
                else:
                    nc.scalar.copy(ob[:], po[:])
                if m >= 12:
                    # tail tiles: split across both queue engines so the
                    # final drain is half a tile, not a whole one
                    nc.sync.dma_start(out=out_d[m * 128:m * 128 + 64, :],
                                      in_=ob[0:64, :])
                    nc.gpsimd.dma_start(out=out_d[m * 128 + 64:(m + 1) * 128, :],
                                        in_=ob[64:128, :])
                else:
                    deng = nc.sync if i % 2 == 0 else nc.gpsimd
                    deng.dma_start(out=out_d[m * 128:(m + 1) * 128, :],
                                   in_=ob[:])

            for j in (1, 0, 3, 2):
                items = head_items(j)
                if j == 1:
                    genmap = edf_schedule(items, head1_chunks(items))
                elif j == 0:
                    genmap = edf_schedule(items, head0_chunks(items))
                else:
                    genmap = {}
                for w in range(4):
                    first_mm[(j, w)] = True
                cur["stage"] = pp.tile([65, S], F32, tag="stage", bufs=2,
                                       name=f"stage{j}")
                if j % 2 == 1:
                    cur["odd"] = pp.tile([64, S], F32R, tag="odd", bufs=1,
                                         name=f"odd{j}")
                pa = {}
                for i, (g, ik) in enumerate(items):
                    if g not in pa:
                        pa[g] = ps.tile([65, 1024], F32, tag="pa", bufs=1,
                                        name=f"pa{j}{g}")
                    fns = genmap.get(i, [])
                    gen = (lambda fns=fns: [f() for f in fns]) if fns else None
                    attn_iter(j, g, ik, pa[g], gen=gen)
                flush_pending()
                for w in range(4):
                    first_mm.pop((j, w), None)

            # ---- projection + output ----
            # flush all chains except the last head's g=1 (interleaved below)
            force_deferred(keep=(2, 1))
            last_chain = [d for d in deferred if d[2] == (2, 1)]
            deferred.clear()
            for m in range(16):
                if last_chain:
                    last_chain.pop(0)[1]()
                if m == 8:
                    while last_chain:
                        last_chain.pop(0)[1]()
                emit_proj(m)
            while last_chain:
                last_chain.pop(0)[1]()

    # consume first_mm flags at first-visible ik
    nc.compile()
    return nc


def _host_prep(x, w_qkv, w_proj):
    """Per-core input slices, packed 128-partition-major and contiguous.
    x/wqk/wv in bf16, wpr in f32."""
    # xT_sl[b]: [128, 4*8*512] slice-major then k-tile-major
    xT_sl = []
    for b in range(B):
        xT = x[b].T.astype(bfloat16)  # [D, S]
        arr = np.empty((128, 4 * KT * 512), bfloat16)
        for s in range(4):
            for k in range(KT):
                arr[:, s * 4096 + k * 512:s * 4096 + (k + 1) * 512] = \
                    xT[k * 128:(k + 1) * 128, s * 512:(s + 1) * 512]
        xT_sl.append(np.ascontiguousarray(arr))
    in_maps = []
    for c in range(N_CORES):
        b, grp = c // 4, c % 4
        heads = list(range(grp * HPC, (grp + 1) * HPC))
        wqk = np.empty((D, 2 * HPC * HD), np.float32)
        wv = np.zeros((D, VW), np.float32)
        wpr = np.empty((HPC * HD, D), np.float32)
        for j, h in enumerate(heads):
            p, i = j // 2, j % 2  # pair, index in pair
            # pair block: [q_a|q_b][k_a|k_b] at 256*p
            wqk[:, p * 256 + i * HD:p * 256 + (i + 1) * HD] = \
                w_qkv[:, h * HD:(h + 1) * HD]
            wqk[:, p * 256 + 128 + i * HD:p * 256 + 128 + (i + 1) * HD] = \
                w_qkv[:, D + h * HD:D + (h + 1) * HD]
            wv[:, j * (HD + 1):j * (HD + 1) + HD] = \
                w_qkv[:, 2 * D + h * HD:2 * D + (h + 1) * HD]
            wpr[j * HD:(j + 1) * HD, :] = w_proj[h * HD:(h + 1) * HD, :]
        # repack k-tile-major [128, KT*cols]
        wqk_sl = np.empty((128, KT * 512), bfloat16)
        wv_sl = np.empty((128, KT * VW), bfloat16)
        for k in range(KT):
            wqk_sl[:, k * 512:(k + 1) * 512] = \
                wqk[k * 128:(k + 1) * 128, :].astype(bfloat16)
            wv_sl[:, k * VW:(k + 1) * VW] = \
                wv[k * 128:(k + 1) * 128, :].astype(bfloat16)
        in_maps.append({
            "xT": xT_sl[b],
            "wqk": np.ascontiguousarray(wqk_sl),
            "wv": np.ascontiguousarray(wv_sl),
            "wpr": np.ascontiguousarray(wpr),
        })
    return in_maps


def get_program(block_mask: np.ndarray):
    key = np.asarray(block_mask, bool).tobytes()
    if key not in _program_cache:
        _program_cache[key] = _build_program(np.asarray(block_mask, bool))
    return _program_cache[key]


def kernel(x, w_qkv, w_proj, b_proj, block_mask):
    x = np.asarray(x, np.float32)
    w_qkv = np.asarray(w_qkv, np.float32)
    w_proj = np.asarray(w_proj, np.float32)
    b_proj = np.asarray(b_proj, np.float32)
    nc = get_program(block_mask)
    in_maps = _host_prep(x, w_qkv, w_proj)
    res = run_bass_kernel_spmd(nc, in_maps, core_ids=list(range(N_CORES)))
    out = np.empty((B, S, D), np.float32)
    for b in range(B):
        acc = np.asarray(res.results[4 * b]["out"], np.float64)
        for g in range(1, 4):
            acc = acc + np.asarray(res.results[4 * b + g]["out"], np.float64)
        out[b] = (acc + b_proj).astype(np.float32)
    return out


# revision 55
# speedup vs baseline: 1.0296x; 1.0265x over previous
"""Block-sparse multi-head attention on 8 Trainium2 NeuronCores.

Problem: y = proj(softmax(mask(q @ k^T / sqrt(hd))) @ v) for
B=2, S=2048, D=1024, H=16 heads, block size 128, with a [16,16] boolean
block mask (True = masked) applied to strictly-upper (k-block > q-block)
blocks.

Sharding: batch x head-group. Core c handles batch c//4 and heads
[4*(c%4), 4*(c%4)+4). No collectives: the host pre-slices inputs
(including pre-transposing x to x^T) and sums the 4 per-batch partial
projection outputs on the way out.

This version fuses all phases into one software-pipelined instruction
stream to keep ScalarE (the exp bottleneck, ~100us/core) and the PE
(~123us/core) simultaneously busy:
  - x/w_qkv/w_v are uploaded in bf16 (halves input DMA to ~6.5MB);
    DMAs are chunked by xT column-slice and issued in consumption order
    so the first attention exp lands ~8us into the kernel.
  - qk-gen for head pair 0 runs first; v-gen and pair-1 qk-gen chunks
    are interleaved into the attention pipeline of heads 1 and 0
    (sharing one PSUM ring) so the PE never idles long enough for HAM
    to re-throttle the clock.
  - attention per head runs as two window passes g=0/1 (pa [65,1024]
    PSUM x2-ring), per k-block: S^T = kpad_ik @ q^T (runs), P~^T =
    exp(S^T/8) (ScalarE, one op per (ik, 1024-window)), PV accumulated
    into pa with the ones-column denominator trick (row 64).
  - normalization: only the two PSUM->SBUF copies are eager; the
    reciprocal/broadcast/multiply chain (which round-trips SBUF DMAs)
    is deferred and spread over the next head's iterations so it never
    head-of-line-blocks the in-order Vector/GpSimd queues.
  - projection is a 4-deep PSUM pipeline (alternating ring slots) with
    PSUM->SBUF copies alternating Vector/Scalar and per-tile output
    DMAs; m-tiles ordered so the last head's deferred normalize chain
    overlaps the first half of proj.
"""

import numpy as np
from ml_dtypes import bfloat16

import concourse.mybir as mybir
import concourse.tile as tile
from concourse import bacc
from concourse.bass_utils import run_bass_kernel_spmd

B, S, D, H = 2, 2048, 1024, 16
HD = 64          # head dim
BS = 128         # mask block size
NB = S // BS     # 16 blocks per axis
HPC = 4          # heads per core
N_CORES = 8
SCALE = HD ** -0.5
KT = D // 128    # 8 k-tiles over the embedding dim
VW = HPC * (HD + 1)  # 260

F32 = mybir.dt.float32
F32R = mybir.dt.float32r
BF16 = mybir.dt.bfloat16
EXP = mybir.ActivationFunctionType.Exp

_program_cache: dict[bytes, object] = {}


def _plan_runs_g(vis, last_vis, ik, g):
    """Contiguous visible q-block runs for k-block ik within 1024-col
    window g. Runs break at 4-block (512-col = PSUM bank) boundaries."""
    runs = []
    jq, end = 8 * g, 8 * g + 8
    while jq < end:
        if not vis[jq][ik]:
            jq += 1
            continue
        start = jq
        while jq + 1 < end and vis[jq + 1][ik] and (jq + 1) % 4 != 0:
            jq += 1
        stopf = any(last_vis[b] == ik for b in range(start, jq + 1))
        runs.append((start, jq - start + 1, stopf))
        jq += 1
    return runs


def _build_program(mask: np.ndarray):
    vis = [[ik <= jq or not bool(mask[jq, ik]) for ik in range(NB)]
           for jq in range(NB)]
    last_vis = [max(ik for ik in range(NB) if vis[jq][ik]) for jq in range(NB)]
    lastw = [max(last_vis[w * 4:(w + 1) * 4]) for w in range(4)]
    RUNS = {(g, ik): _plan_runs_g(vis, last_vis, ik, g)
            for g in range(2) for ik in range(NB)}

    nc = bacc.Bacc("TRN2", target_bir_lowering=False, debug=False,
                   num_devices=N_CORES)
    # host pre-packs everything 128-partition-major and fully contiguous:
    # xT_sl: [128, 4*8*512]  slice-major: slice s (512 seq cols), then k-tile
    # wqk_sl: [128, 8*512]   k-tile major; within: [q0|q1][k0|k1][q2|q3][k2|k3]
    # wv_sl:  [128, 8*260]   k-tile major
    xT_d = nc.dram_tensor("xT", [128, 4 * KT * 512], BF16, kind="ExternalInput")
    wqk_d = nc.dram_tensor("wqk", [128, KT * 512], BF16, kind="ExternalInput")
    wv_d = nc.dram_tensor("wv", [128, KT * VW], BF16, kind="ExternalInput")
    wpr_d = nc.dram_tensor("wpr", [HPC * HD, D], F32R, kind="ExternalInput")
    out_d = nc.dram_tensor("out", [S, D], BF16, kind="ExternalOutput")

    with tile.TileContext(nc) as tc:
        with tc.tile_pool(name="pp", bufs=1) as pp, \
             tc.tile_pool(name="ptp", bufs=5) as ptp, \
             tc.tile_pool(name="ps", bufs=2, space="PSUM") as ps:
            # ---- persistent SBUF tiles ----
            xT_sl = pp.tile([128, 4 * KT * 512], BF16, tag="xT", name="xT")
            wqk_sl = pp.tile([128, KT * 512], BF16, tag="wqk", name="wqk")
            wv_sl = pp.tile([128, KT * VW], BF16, tag="wv", name="wv")
            wpr_t = [pp.tile([128, D], F32R, tag=f"wpr{k}", name=f"wpr{k}")
                     for k in range(2)]
            q_t = [pp.tile([128, S], BF16, tag=f"q{p}", name=f"q{p}")
                   for p in range(2)]
            kpad_t = [pp.tile([128, S], BF16, tag=f"kp{h}", name=f"kp{h}")
                      for h in range(HPC)]
            v_t = [pp.tile([128, VW], F32R, tag=f"v{m}", name=f"v{m}")
                   for m in range(NB)]
            attn_t = [pp.tile([128, S], F32R, tag=f"attn{i}", name=f"attn{i}")
                      for i in range(2)]
            d16_t = pp.tile([128, 8 * HPC * 2], F32, tag="d16", name="d16")
            r0_t = pp.tile([1, S], F32, tag="r0", name="r0")
            onec = pp.tile([128, 4], F32, tag="onec", name="onec")
            zsrc = pp.tile([64, 512], F32, tag="zsrc", name="zsrc")
            scr = pp.tile([128, 4], F32, tag="scr", name="scr")

            # ---- init + ACT table pre-warm ----
            nc.vector.memset(onec[:], 1.0)
            nc.vector.memset(zsrc[:], 0.0)
            nc.scalar.activation(scr[:], onec[:], EXP, scale=1.0)
            for h in range(HPC):
                z0 = 64 if h % 2 == 0 else 0
                for c in range(4):
                    eng = nc.vector if (h * 4 + c) % 2 == 0 else nc.scalar
                    cs = c * 512
                    if eng is nc.vector:
                        eng.tensor_copy(kpad_t[h][z0:z0 + 64, cs:cs + 512],
                                        zsrc[:])
                    else:
                        eng.copy(kpad_t[h][z0:z0 + 64, cs:cs + 512], zsrc[:])

            # ---- input DMAs: few big contiguous pieces, consumption order,
            # issue alternating between the sync and gpsimd queues (descriptor
            # generation is ~0.6us each and serial per queue) ----
            dmact = [0]

            def in_dma(dst, src):
                eng = nc.sync if dmact[0] % 2 == 0 else nc.gpsimd
                dmact[0] += 1
                eng.dma_start(out=dst, in_=src)

            for h in range(4):  # wqk + xT s0 interleaved: 8 x 256KB
                o = h * 1024
                in_dma(wqk_sl[:, o:o + 1024], wqk_d[:, o:o + 1024])
                in_dma(xT_sl[:, o:o + 1024], xT_d[:, o:o + 1024])
            for h in range(2):  # xT s1: 2 x 512KB
                o = 4096 + h * 2048
                in_dma(xT_sl[:, o:o + 2048], xT_d[:, o:o + 2048])
            hw = KT * VW // 2
            for h in range(2):  # wv: 2 x 265KB
                in_dma(wv_sl[:, h * hw:(h + 1) * hw],
                       wv_d[:, h * hw:(h + 1) * hw])
            for p4 in range(4):  # xT s2+s3: 4 x 512KB
                o = 2 * 4096 + p4 * 2048
                in_dma(xT_sl[:, o:o + 2048], xT_d[:, o:o + 2048])
            for k in range(2):
                in_dma(wpr_t[k][:], wpr_d[k * 128:(k + 1) * 128, :])

            # ---- gen chunk emitters (copies alternate Vector/Scalar) ----
            genct = [0]

            def qk_chunk(p, t, c, lead=False):
                """[128,512] chunk of q-pair (t=0) or k-pair (t=1) tile."""
                pb = ps.tile([128, 512], F32, tag="st", bufs=3, name=f"pb{p}{t}{c}")
                off = p * 256 + t * 128
                cs = c * 512
                for k in range(KT):
                    nc.tensor.matmul(
                        pb[:], wqk_sl[:, k * 512 + off:k * 512 + off + 128],
                        xT_sl[:, c * 4096 + k * 512:c * 4096 + (k + 1) * 512],
                        start=(k == 0), stop=(k == KT - 1))
                genct[0] += 1
                use_sc = genct[0] % 2 == 1
                # keep both half-copies of one chunk on ONE engine: the
                # framework serializes sibling readers cross-engine, which
                # couples the exp stream to the Vector queue otherwise
                if t == 0:
                    if use_sc:
                        nc.scalar.copy(q_t[p][:, cs:cs + 512], pb[:])
                    else:
                        nc.vector.tensor_copy(q_t[p][:, cs:cs + 512], pb[:])
                else:
                    h0, h1 = 2 * p, 2 * p + 1
                    if use_sc:
                        nc.scalar.copy(kpad_t[h0][0:64, cs:cs + 512],
                                       pb[0:64, :])
                        nc.scalar.copy(kpad_t[h1][64:128, cs:cs + 512],
                                       pb[64:128, :])
                    else:
                        nc.vector.tensor_copy(kpad_t[h0][0:64, cs:cs + 512],
                                              pb[0:64, :])
                        nc.vector.tensor_copy(kpad_t[h1][64:128, cs:cs + 512],
                                              pb[64:128, :])

            def v_chunk(m):
                pc = ps.tile([128, 512], F32, tag="st", bufs=3, name=f"pc{m}")
                s, r = m // 4, m % 4
                for k in range(KT):
                    nc.tensor.matmul(
                        pc[:, 0:VW],
                        xT_sl[:, s * 4096 + k * 512 + r * 128:
                               s * 4096 + k * 512 + (r + 1) * 128],
                        wv_sl[:, k * VW:(k + 1) * VW],
                        start=(k == 0), stop=(k == KT - 1))
                nc.vector.tensor_copy(v_t[m][:], pc[:, 0:VW])
                nc.vector.tensor_copy(v_t[m][:, HD::HD + 1], onec[:])

            # ---- deferred-op machinery ----
            deferred = []  # [countdown, fn]

            def poll_deferred():
                due = [d for d in deferred if d[0] <= 1]
                for d in due:
                    deferred.remove(d)
                for d in deferred:
                    d[0] -= 1
                for d in due:
                    d[1]()

            def force_deferred(keep=None):
                kept = []
                while deferred:
                    d = deferred.pop(0)
                    if keep is not None and d[2] == keep:
                        kept.append(d)
                    else:
                        d[1]()
                deferred.extend(kept)

            # ---- normalize chain ----
            first_mm = {}   # (j, w) -> True once consumed
            wins_done = {}  # (j, g) -> count

            # per-head staging, ring-allocated (lifetimes span into next head).
            # stage[0:64] = unnormalized attn rows, stage[64:65] = denominator.
            cur = {"stage": None, "odd": None}

            def enqueue_chain(j, g, spacing):
                p, gc = j // 2, g * 1024
                sl = d16_t[:, (2 * j + g) * 8:(2 * j + g + 1) * 8]
                stage, odd = cur["stage"], cur["odd"]
                if j % 2 == 0:
                    dst = attn_t[p][0:64, gc:gc + 1024]
                else:
                    dst = odd[0:64, gc:gc + 1024]

                def s1():
                    nc.gpsimd.dma_start(out=sl, in_=stage[64:65, gc:gc + 1024])

                def s2():
                    nc.vector.reciprocal(sl, sl)

                def s3():
                    nc.gpsimd.dma_start(out=r0_t[0:1, gc:gc + 1024], in_=sl)

                def s4(h):
                    hc = gc + h * 512
                    dbc = pp.tile([64, 512], F32, tag="dbc", bufs=4,
                                  name=f"dbc{j}{g}{h}")
                    cur[f"dbc{j}{g}{h}"] = dbc
                    nc.gpsimd.partition_broadcast(dbc[:],
                                                  r0_t[0:1, hc:hc + 512])

                def s5(h):
                    hc = gc + h * 512
                    dbc = cur.pop(f"dbc{j}{g}{h}")
                    nc.vector.tensor_mul(dst[:, h * 512:(h + 1) * 512],
                                         stage[0:64, hc:hc + 512], dbc[:])

                def s6():
                    nc.gpsimd.dma_start(out=attn_t[p][64:128, gc:gc + 1024],
                                        in_=odd[0:64, gc:gc + 1024])

                # both PBs issued before the MULs so the GpSimd latency is
                # hidden before the Vector ops need the result
                steps = [(1, s1), (2, s2), (1, s3),
                         (1, lambda: s4(0)), (1, lambda: s4(1)),
                         (2, lambda: s5(0)), (1, lambda: s5(1))]
                if j % 2 == 1:
                    steps.append((1, s6))
                cd = 0
                for extra, fn in steps:
                    cd += spacing * extra
                    deferred.append([cd, fn, (j, g)])

            def norm_copies(j, g, ik, pa_g):
                for w in (2 * g, 2 * g + 1):
                    if lastw[w] != ik:
                        continue
                    ws = w * 512
                    rel = ws - g * 1024
                    nc.vector.tensor_copy(cur["stage"][0:65, ws:ws + 512],
                                          pa_g[0:65, rel:rel + 512])
                    wins_done[(j, g)] = wins_done.get((j, g), 0) + 1
                    if wins_done[(j, g)] == 2:
                        enqueue_chain(j, g, spacing=(1 if j == 2 else 2))

            # ---- attention pipeline ----
            pending = [None]  # [(j, g, ik, runs, ptg, pa_g)]

            def flush_pending():
                item = pending[0]
                pending[0] = None
                if item is None:
                    return
                j, g, ik, runs, ptg, pa_g = item
                lhsT_v = v_t[ik][:, j * (HD + 1):(j + 1) * (HD + 1)]
                for (qb0, nbk, stopf) in runs:
                    qs, qlen = qb0 * 128, nbk * 128
                    rel = qs - g * 1024
                    w = qb0 // 4
                    startf = first_mm.pop((j, w), False)
                    nc.tensor.matmul(pa_g[0:65, rel:rel + qlen], lhsT_v,
                                     ptg[:, rel:rel + qlen],
                                     start=startf, stop=stopf,
                                     skip_group_check=True)
                norm_copies(j, g, ik, pa_g)

            def attn_iter(j, g, ik, pa_g, gen=None):
                runs = RUNS[(g, ik)]
                stg = ps.tile([128, 1024], F32, tag="st", bufs=3, name=f"st{j}{g}{ik}")
                lhsT_k = kpad_t[j][:, ik * 128:(ik + 1) * 128]
                qtile = q_t[j // 2]
                for (qb0, nbk, stopf) in runs:
                    qs, qlen = qb0 * 128, nbk * 128
                    rel = qs - g * 1024
                    nc.tensor.matmul(stg[:, rel:rel + qlen], lhsT_k,
                                     qtile[:, qs:qs + qlen],
                                     start=True, stop=True)
                # one exp op per cluster of runs; split where the masked gap
                # exceeds 512 cols (gap cols cost more than a second op)
                ptg = ptp.tile([128, 1024], F32R, tag="pt", name=f"pt{j}{g}{ik}")
                clusters = []
                for (qb0, nbk, _s) in runs:
                    rlo = qb0 * 128 - g * 1024
                    rhi = rlo + nbk * 128
                    if clusters and rlo - clusters[-1][1] <= 512:
                        clusters[-1][1] = rhi
                    else:
                        clusters.append([rlo, rhi])
                for (clo, chi) in clusters:
                    nc.scalar.activation(ptg[:, clo:chi], stg[:, clo:chi], EXP,
                                         scale=SCALE)
                if gen is not None:
                    gen()
                poll_deferred()
                flush_pending()
                pending[0] = (j, g, ik, runs, ptg, pa_g)

            # ---- lead: qk-gen for pair 0, windows g=0 ----
            for (t, c) in ((0, 0), (1, 0), (0, 1), (1, 1)):
                qk_chunk(0, t, c, lead=True)

            iters = {g: [ik for ik in range(NB) if RUNS[(g, ik)]]
                     for g in range(2)}

            def head_items(j):
                # g=0 fully then g=1: only ONE pa tile live at a time, which
                # frees 2 PSUM banks for a 3-deep stg ring (the exp pipeline)
                phases = [(0, iters[0]), (1, iters[1])]
                return [(g, ik) for (g, iklist) in phases for ik in iklist]

            def edf_schedule(items, chunks):
                """Assign gen chunks to iteration slots by earliest deadline.
                chunks: list of (deadline_slot_inclusive, fn). Returns
                slot -> [fns]; infeasible chunks go to slot 0."""
                slots = {i: [] for i in range(len(items))}
                fill = {i: 0 for i in range(len(items))}
                for dl, fn in sorted(chunks, key=lambda c: c[0]):
                    placed = False
                    # latest-fit: emit just-in-time so gen MMs queue behind
                    # already-arrived DMA data instead of stalling the PE
                    for s in range(min(dl, len(items) - 1), -1, -1):
                        if fill[s] < 2:
                            slots[s].append(fn)
                            fill[s] += 1
                            placed = True
                            break
                    if not placed:
                        slots[0].insert(0, fn)
                return slots

            def head1_chunks(items):
                """v tiles (PV deadline) + pair-0 windows g=1 (QK deadline)."""
                chunks = []
                for m in range(NB):
                    idx = min((i for i, (g, ik) in enumerate(items) if ik == m),
                              default=0)
                    chunks.append((idx + 1, lambda m=m: v_chunk(m)))
                for c in (2, 3):
                    # q chunk c: first QK of window-pair g=c//2 touching it
                    idx = min((i for i, (g, ik) in enumerate(items)
                               if g == c // 2), default=1)
                    chunks.append((max(0, idx - 1),
                                   lambda c=c: qk_chunk(0, 0, c)))
                    # kpad chunk c: first QK with ik in [4c, 4c+4)
                    idx = min((i for i, (g, ik) in enumerate(items)
                               if 4 * c <= ik < 4 * c + 4), default=1)
                    chunks.append((max(0, idx - 1),
                                   lambda c=c: qk_chunk(0, 1, c)))
                return chunks

            def head0_chunks(items):
                """pair-1 gen, needed only by heads 3/2: spread evenly."""
                chunks = []
                pos = 0
                for c in range(4):
                    for t in (0, 1):
                        chunks.append((pos, lambda t=t, c=c: qk_chunk(1, t, c)))
                        pos += 3
                return chunks

            # ---- projection tile emitter (used by proj loop AND as PE
            # filler in head 2's g=1 pass, once g0 windows are final) ----
            projct = [0]

            def emit_proj(m):
                i = projct[0]
                projct[0] += 1
                po = ps.tile([128, D], F32, tag="st", bufs=3, name=f"po{m}")
                for kt in range(2):
                    for c in range(2):
                        nc.tensor.matmul(
                            po[:, c * 512:(c + 1) * 512],
                            attn_t[kt][:, m * 128:(m + 1) * 128],
                            wpr_t[kt][:, c * 512:(c + 1) * 512],
                            start=(kt == 0), stop=(kt == 1))
                ob = pp.tile([128, D], BF16, tag="ob", bufs=3, name=f"ob{m}")
                if i % 2 == 0:
                    nc.vector.tensor_copy(ob[:], po[:])
                else:
                    nc.scalar.copy(ob[:], po[:])
                if m >= 12:
                    # tail tiles: split across both queue engines so the
                    # final drain is half a tile, not a whole one
                    nc.sync.dma_start(out=out_d[m * 128:m * 128 + 64, :],
                                      in_=ob[0:64, :])
                    nc.gpsimd.dma_start(out=out_d[m * 128 + 64:(m + 1) * 128, :],
                                        in_=ob[64:128, :])
                else:
                    deng = nc.sync if i % 2 == 0 else nc.gpsimd
                    deng.dma_start(out=out_d[m * 128:(m + 1) * 128, :],
                                   in_=ob[:])

            for j in (1, 0, 3, 2):
                items = head_items(j)
                if j == 1:
                    genmap = edf_schedule(items, head1_chunks(items))
                elif j == 0:
                    genmap = edf_schedule(items, head0_chunks(items))
                else:
                    genmap = {}
                for w in range(4):
                    first_mm[(j, w)] = True
                cur["stage"] = pp.tile([65, S], F32, tag="stage", bufs=2,
                                       name=f"stage{j}")
                if j % 2 == 1:
                    cur["odd"] = pp.tile([64, S], F32R, tag="odd", bufs=1,
                                         name=f"odd{j}")
                pa = {}
                for i, (g, ik) in enumerate(items):
                    if g not in pa:
                        pa[g] = ps.tile([65, 1024], F32, tag="pa", bufs=1,
                                        name=f"pa{j}{g}")
                    fns = genmap.get(i, [])
                    gen = (lambda fns=fns: [f() for f in fns]) if fns else None
                    attn_iter(j, g, ik, pa[g], gen=gen)
                flush_pending()
                for w in range(4):
                    first_mm.pop((j, w), None)

            # ---- projection + output ----
            # flush all chains except the last head's g=1 (interleaved below)
            force_deferred(keep=(2, 1))
            last_chain = [d for d in deferred if d[2] == (2, 1)]
            deferred.clear()
            for m in range(16):
                if last_chain:
                    last_chain.pop(0)[1]()
                if m == 8:
                    while last_chain:
                        last_chain.pop(0)[1]()
                emit_proj(m)
            while last_chain:
                last_chain.pop(0)[1]()

    # consume first_mm flags at first-visible ik
    nc.compile()
    return nc


def _host_prep(x, w_qkv, w_proj):
    """Per-core input slices, packed 128-partition-major and contiguous.
    x/wqk/wv in bf16, wpr in f32."""
    # xT_sl[b]: [128, 4*8*512] slice-major then k-tile-major
    xT_sl = []
    for b in range(B):
        xT = x[b].T.astype(bfloat16)  # [D, S]
        arr = np.empty((128, 4 * KT * 512), bfloat16)
        for s in range(4):
            for k in range(KT):
                arr[:, s * 4096 + k * 512:s * 4096 + (k + 1) * 512] = \
                    xT[k * 128:(k + 1) * 128, s * 512:(s + 1) * 512]
        xT_sl.append(np.ascontiguousarray(arr))
    in_maps = []
    for c in range(N_CORES):
        b, grp = c // 4, c % 4
        heads = list(range(grp * HPC, (grp + 1) * HPC))
        wqk = np.empty((D, 2 * HPC * HD), np.float32)
        wv = np.zeros((D, VW), np.float32)
        wpr = np.empty((HPC * HD, D), np.float32)
        for j, h in enumerate(heads):
            p, i = j // 2, j % 2  # pair, index in pair
            # pair block: [q_a|q_b][k_a|k_b] at 256*p
            wqk[:, p * 256 + i * HD:p * 256 + (i + 1) * HD] = \
                w_qkv[:, h * HD:(h + 1) * HD]
            wqk[:, p * 256 + 128 + i * HD:p * 256 + 128 + (i + 1) * HD] = \
                w_qkv[:, D + h * HD:D + (h + 1) * HD]
            wv[:, j * (HD + 1):j * (HD + 1) + HD] = \
                w_qkv[:, 2 * D + h * HD:2 * D + (h + 1) * HD]
            wpr[j * HD:(j + 1) * HD, :] = w_proj[h * HD:(h + 1) * HD, :]
        # repack k-tile-major [128, KT*cols]
        wqk_sl = np.empty((128, KT * 512), bfloat16)
        wv_sl = np.empty((128, KT * VW), bfloat16)
        for k in range(KT):
            wqk_sl[:, k * 512:(k + 1) * 512] = \
                wqk[k * 128:(k + 1) * 128, :].astype(bfloat16)
            wv_sl[:, k * VW:(k + 1) * VW] = \
                wv[k * 128:(k + 1) * 128, :].astype(bfloat16)
        in_maps.append({
            "xT": xT_sl[b],
            "wqk": np.ascontiguousarray(wqk_sl),
            "wv": np.ascontiguousarray(wv_sl),
            "wpr": np.ascontiguousarray(wpr),
        })
    return in_maps


def get_program(block_mask: np.ndarray):
    key = np.asarray(block_mask, bool).tobytes()
    if key not in _program_cache:
        _program_cache[key] = _build_program(np.asarray(block_mask, bool))
    return _program_cache[key]


def kernel(x, w_qkv, w_proj, b_proj, block_mask):
    x = np.asarray(x, np.float32)
    w_qkv = np.asarray(w_qkv, np.float32)
    w_proj = np.asarray(w_proj, np.float32)
    b_proj = np.asarray(b_proj, np.float32)
    nc = get_program(block_mask)
    in_maps = _host_prep(x, w_qkv, w_proj)
    res = run_bass_kernel_spmd(nc, in_maps, core_ids=list(range(N_CORES)))
    out = np.empty((B, S, D), np.float32)
    for b in range(B):
        acc = np.asarray(res.results[4 * b]["out"], np.float64)
        for g in range(1, 4):
            acc = acc + np.asarray(res.results[4 * b + g]["out"], np.float64)
        out[b] = (acc + b_proj).astype(np.float32)
    return out


# revision 56
# speedup vs baseline: 1.0447x; 1.0146x over previous
"""Block-sparse multi-head attention on 8 Trainium2 NeuronCores.

Problem: y = proj(softmax(mask(q @ k^T / sqrt(hd))) @ v) for
B=2, S=2048, D=1024, H=16 heads, block size 128, with a [16,16] boolean
block mask (True = masked) applied to strictly-upper (k-block > q-block)
blocks.

Sharding: batch x head-group. Core c handles batch c//4 and heads
[4*(c%4), 4*(c%4)+4). No collectives: the host pre-slices inputs
(including pre-transposing x to x^T) and sums the 4 per-batch partial
projection outputs on the way out.

This version fuses all phases into one software-pipelined instruction
stream to keep ScalarE (the exp bottleneck, ~100us/core) and the PE
(~123us/core) simultaneously busy:
  - x/w_qkv/w_v are uploaded in bf16 (halves input DMA to ~6.5MB);
    DMAs are chunked by xT column-slice and issued in consumption order
    so the first attention exp lands ~8us into the kernel.
  - qk-gen for head pair 0 runs first; v-gen and pair-1 qk-gen chunks
    are interleaved into the attention pipeline of heads 1 and 0
    (sharing one PSUM ring) so the PE never idles long enough for HAM
    to re-throttle the clock.
  - attention per head runs as two window passes g=0/1 (pa [65,1024]
    PSUM x2-ring), per k-block: S^T = kpad_ik @ q^T (runs), P~^T =
    exp(S^T/8) (ScalarE, one op per (ik, 1024-window)), PV accumulated
    into pa with the ones-column denominator trick (row 64).
  - normalization: only the two PSUM->SBUF copies are eager; the
    reciprocal/broadcast/multiply chain (which round-trips SBUF DMAs)
    is deferred and spread over the next head's iterations so it never
    head-of-line-blocks the in-order Vector/GpSimd queues.
  - projection is a 4-deep PSUM pipeline (alternating ring slots) with
    PSUM->SBUF copies alternating Vector/Scalar and per-tile output
    DMAs; m-tiles ordered so the last head's deferred normalize chain
    overlaps the first half of proj.
"""

import numpy as np
from ml_dtypes import bfloat16

import concourse.mybir as mybir
import concourse.tile as tile
from concourse import bacc
from concourse.bass_utils import run_bass_kernel_spmd

B, S, D, H = 2, 2048, 1024, 16
HD = 64          # head dim
BS = 128         # mask block size
NB = S // BS     # 16 blocks per axis
HPC = 4          # heads per core
N_CORES = 8
SCALE = HD ** -0.5
KT = D // 128    # 8 k-tiles over the embedding dim
VW = HPC * (HD + 1)  # 260

F32 = mybir.dt.float32
F32R = mybir.dt.float32r
BF16 = mybir.dt.bfloat16
EXP = mybir.ActivationFunctionType.Exp

_program_cache: dict[bytes, object] = {}


def _plan_runs_g(vis, last_vis, ik, g):
    """Contiguous visible q-block runs for k-block ik within 1024-col
    window g. Runs break at 4-block (512-col = PSUM bank) boundaries."""
    runs = []
    jq, end = 8 * g, 8 * g + 8
    while jq < end:
        if not vis[jq][ik]:
            jq += 1
            continue
        start = jq
        while jq + 1 < end and vis[jq + 1][ik] and (jq + 1) % 4 != 0:
            jq += 1
        stopf = any(last_vis[b] == ik for b in range(start, jq + 1))
        runs.append((start, jq - start + 1, stopf))
        jq += 1
    return runs


def _build_program(mask: np.ndarray):
    vis = [[ik <= jq or not bool(mask[jq, ik]) for ik in range(NB)]
           for jq in range(NB)]
    last_vis = [max(ik for ik in range(NB) if vis[jq][ik]) for jq in range(NB)]
    lastw = [max(last_vis[w * 4:(w + 1) * 4]) for w in range(4)]
    RUNS = {(g, ik): _plan_runs_g(vis, last_vis, ik, g)
            for g in range(2) for ik in range(NB)}

    nc = bacc.Bacc("TRN2", target_bir_lowering=False, debug=False,
                   num_devices=N_CORES)
    # host pre-packs everything 128-partition-major and fully contiguous:
    # xT_sl: [128, 4*8*512]  slice-major: slice s (512 seq cols), then k-tile
    # wqk_sl: [128, 8*512]   k-tile major; within: [q0|q1][k0|k1][q2|q3][k2|k3]
    # wv_sl:  [128, 8*260]   k-tile major
    xT_d = nc.dram_tensor("xT", [128, 4 * KT * 512], BF16, kind="ExternalInput")
    wqk_d = nc.dram_tensor("wqk", [128, KT * 512], BF16, kind="ExternalInput")
    wv_d = nc.dram_tensor("wv", [128, KT * VW], BF16, kind="ExternalInput")
    wpr_d = nc.dram_tensor("wpr", [HPC * HD, D], BF16, kind="ExternalInput")
    out_d = nc.dram_tensor("out", [S, D], BF16, kind="ExternalOutput")

    with tile.TileContext(nc) as tc:
        with tc.tile_pool(name="pp", bufs=1) as pp, \
             tc.tile_pool(name="ptp", bufs=5) as ptp, \
             tc.tile_pool(name="ps", bufs=2, space="PSUM") as ps:
            # ---- persistent SBUF tiles ----
            xT_sl = pp.tile([128, 4 * KT * 512], BF16, tag="xT", name="xT")
            wqk_sl = pp.tile([128, KT * 512], BF16, tag="wqk", name="wqk")
            wv_sl = pp.tile([128, KT * VW], BF16, tag="wv", name="wv")
            wpr_t = [pp.tile([128, D], BF16, tag=f"wpr{k}", name=f"wpr{k}")
                     for k in range(2)]
            q_t = [pp.tile([128, S], BF16, tag=f"q{p}", name=f"q{p}")
                   for p in range(2)]
            kpad_t = [pp.tile([128, S], BF16, tag=f"kp{h}", name=f"kp{h}")
                      for h in range(HPC)]
            v_t = [pp.tile([128, VW], F32R, tag=f"v{m}", name=f"v{m}")
                   for m in range(NB)]
            attn_t = [pp.tile([128, S], BF16, tag=f"attn{i}", name=f"attn{i}")
                      for i in range(2)]
            d16_t = pp.tile([128, 8 * HPC * 2], F32, tag="d16", name="d16")
            r0_t = pp.tile([1, S], F32, tag="r0", name="r0")
            onec = pp.tile([128, 4], F32, tag="onec", name="onec")
            zsrc = pp.tile([64, 512], F32, tag="zsrc", name="zsrc")
            scr = pp.tile([128, 4], F32, tag="scr", name="scr")

            # ---- init + ACT table pre-warm ----
            nc.vector.memset(onec[:], 1.0)
            nc.vector.memset(zsrc[:], 0.0)
            nc.scalar.activation(scr[:], onec[:], EXP, scale=1.0)
            for h in range(HPC):
                z0 = 64 if h % 2 == 0 else 0
                for c in range(4):
                    eng = nc.vector if (h * 4 + c) % 2 == 0 else nc.scalar
                    cs = c * 512
                    if eng is nc.vector:
                        eng.tensor_copy(kpad_t[h][z0:z0 + 64, cs:cs + 512],
                                        zsrc[:])
                    else:
                        eng.copy(kpad_t[h][z0:z0 + 64, cs:cs + 512], zsrc[:])

            # ---- input DMAs: few big contiguous pieces, consumption order,
            # issue alternating between the sync and gpsimd queues (descriptor
            # generation is ~0.6us each and serial per queue) ----
            dmact = [0]

            def in_dma(dst, src):
                eng = nc.sync if dmact[0] % 2 == 0 else nc.gpsimd
                dmact[0] += 1
                eng.dma_start(out=dst, in_=src)

            for h in range(4):  # wqk + xT s0 interleaved: 8 x 256KB
                o = h * 1024
                in_dma(wqk_sl[:, o:o + 1024], wqk_d[:, o:o + 1024])
                in_dma(xT_sl[:, o:o + 1024], xT_d[:, o:o + 1024])
            for h in range(2):  # xT s1: 2 x 512KB
                o = 4096 + h * 2048
                in_dma(xT_sl[:, o:o + 2048], xT_d[:, o:o + 2048])
            hw = KT * VW // 2
            for h in range(2):  # wv: 2 x 265KB
                in_dma(wv_sl[:, h * hw:(h + 1) * hw],
                       wv_d[:, h * hw:(h + 1) * hw])
            for p4 in range(4):  # xT s2+s3: 4 x 512KB
                o = 2 * 4096 + p4 * 2048
                in_dma(xT_sl[:, o:o + 2048], xT_d[:, o:o + 2048])
            for k in range(2):
                in_dma(wpr_t[k][:], wpr_d[k * 128:(k + 1) * 128, :])

            # ---- gen chunk emitters (copies alternate Vector/Scalar) ----
            genct = [0]

            def qk_chunk(p, t, c, lead=False):
                """[128,512] chunk of q-pair (t=0) or k-pair (t=1) tile."""
                pb = ps.tile([128, 512], F32, tag="st", bufs=3, name=f"pb{p}{t}{c}")
                off = p * 256 + t * 128
                cs = c * 512
                for k in range(KT):
                    nc.tensor.matmul(
                        pb[:], wqk_sl[:, k * 512 + off:k * 512 + off + 128],
                        xT_sl[:, c * 4096 + k * 512:c * 4096 + (k + 1) * 512],
                        start=(k == 0), stop=(k == KT - 1))
                genct[0] += 1
                use_sc = genct[0] % 2 == 1
                # keep both half-copies of one chunk on ONE engine: the
                # framework serializes sibling readers cross-engine, which
                # couples the exp stream to the Vector queue otherwise
                if t == 0:
                    if use_sc:
                        nc.scalar.copy(q_t[p][:, cs:cs + 512], pb[:])
                    else:
                        nc.vector.tensor_copy(q_t[p][:, cs:cs + 512], pb[:])
                else:
                    h0, h1 = 2 * p, 2 * p + 1
                    if use_sc:
                        nc.scalar.copy(kpad_t[h0][0:64, cs:cs + 512],
                                       pb[0:64, :])
                        nc.scalar.copy(kpad_t[h1][64:128, cs:cs + 512],
                                       pb[64:128, :])
                    else:
                        nc.vector.tensor_copy(kpad_t[h0][0:64, cs:cs + 512],
                                              pb[0:64, :])
                        nc.vector.tensor_copy(kpad_t[h1][64:128, cs:cs + 512],
                                              pb[64:128, :])

            def v_chunk(m):
                pc = ps.tile([128, 512], F32, tag="st", bufs=3, name=f"pc{m}")
                s, r = m // 4, m % 4
                for k in range(KT):
                    nc.tensor.matmul(
                        pc[:, 0:VW],
                        xT_sl[:, s * 4096 + k * 512 + r * 128:
                               s * 4096 + k * 512 + (r + 1) * 128],
                        wv_sl[:, k * VW:(k + 1) * VW],
                        start=(k == 0), stop=(k == KT - 1))
                nc.vector.tensor_copy(v_t[m][:], pc[:, 0:VW])
                nc.vector.tensor_copy(v_t[m][:, HD::HD + 1], onec[:])

            # ---- deferred-op machinery ----
            deferred = []  # [countdown, fn]

            def poll_deferred():
                due = [d for d in deferred if d[0] <= 1]
                for d in due:
                    deferred.remove(d)
                for d in deferred:
                    d[0] -= 1
                for d in due:
                    d[1]()

            def force_deferred(keep=None):
                kept = []
                while deferred:
                    d = deferred.pop(0)
                    if keep is not None and d[2] == keep:
                        kept.append(d)
                    else:
                        d[1]()
                deferred.extend(kept)

            # ---- normalize chain ----
            first_mm = {}   # (j, w) -> True once consumed
            wins_done = {}  # (j, g) -> count

            # per-head staging, ring-allocated (lifetimes span into next head).
            # stage[0:64] = unnormalized attn rows, stage[64:65] = denominator.
            cur = {"stage": None, "odd": None}

            def enqueue_chain(j, g, spacing):
                p, gc = j // 2, g * 1024
                sl = d16_t[:, (2 * j + g) * 8:(2 * j + g + 1) * 8]
                stage, odd = cur["stage"], cur["odd"]
                if j % 2 == 0:
                    dst = attn_t[p][0:64, gc:gc + 1024]
                else:
                    dst = odd[0:64, gc:gc + 1024]

                def s1():
                    nc.gpsimd.dma_start(out=sl, in_=stage[64:65, gc:gc + 1024])

                def s2():
                    nc.vector.reciprocal(sl, sl)

                def s3():
                    nc.gpsimd.dma_start(out=r0_t[0:1, gc:gc + 1024], in_=sl)

                def s4(h):
                    hc = gc + h * 512
                    dbc = pp.tile([64, 512], F32, tag="dbc", bufs=4,
                                  name=f"dbc{j}{g}{h}")
                    cur[f"dbc{j}{g}{h}"] = dbc
                    nc.gpsimd.partition_broadcast(dbc[:],
                                                  r0_t[0:1, hc:hc + 512])

                def s5(h):
                    hc = gc + h * 512
                    dbc = cur.pop(f"dbc{j}{g}{h}")
                    nc.vector.tensor_mul(dst[:, h * 512:(h + 1) * 512],
                                         stage[0:64, hc:hc + 512], dbc[:])

                def s6():
                    nc.gpsimd.dma_start(out=attn_t[p][64:128, gc:gc + 1024],
                                        in_=odd[0:64, gc:gc + 1024])

                # both PBs issued before the MULs so the GpSimd latency is
                # hidden before the Vector ops need the result
                steps = [(1, s1), (2, s2), (1, s3),
                         (1, lambda: s4(0)), (1, lambda: s4(1)),
                         (2, lambda: s5(0)), (1, lambda: s5(1))]
                if j % 2 == 1:
                    steps.append((1, s6))
                cd = 0
                for extra, fn in steps:
                    cd += spacing * extra
                    deferred.append([cd, fn, (j, g)])

            def norm_copies(j, g, ik, pa_g):
                for w in (2 * g, 2 * g + 1):
                    if lastw[w] != ik:
                        continue
                    ws = w * 512
                    rel = ws - g * 1024
                    nc.vector.tensor_copy(cur["stage"][0:65, ws:ws + 512],
                                          pa_g[0:65, rel:rel + 512])
                    wins_done[(j, g)] = wins_done.get((j, g), 0) + 1
                    if wins_done[(j, g)] == 2:
                        enqueue_chain(j, g, spacing=(1 if j == 2 else 2))

            # ---- attention pipeline ----
            pending = [None]  # [(j, g, ik, runs, ptg, pa_g)]

            def flush_pending():
                item = pending[0]
                pending[0] = None
                if item is None:
                    return
                j, g, ik, runs, ptg, pa_g = item
                lhsT_v = v_t[ik][:, j * (HD + 1):(j + 1) * (HD + 1)]
                for (qb0, nbk, stopf) in runs:
                    qs, qlen = qb0 * 128, nbk * 128
                    rel = qs - g * 1024
                    w = qb0 // 4
                    startf = first_mm.pop((j, w), False)
                    nc.tensor.matmul(pa_g[0:65, rel:rel + qlen], lhsT_v,
                                     ptg[:, rel:rel + qlen],
                                     start=startf, stop=stopf,
                                     skip_group_check=True)
                norm_copies(j, g, ik, pa_g)

            def attn_iter(j, g, ik, pa_g, gen=None):
                runs = RUNS[(g, ik)]
                stg = ps.tile([128, 1024], F32, tag="st", bufs=3, name=f"st{j}{g}{ik}")
                lhsT_k = kpad_t[j][:, ik * 128:(ik + 1) * 128]
                qtile = q_t[j // 2]
                for (qb0, nbk, stopf) in runs:
                    qs, qlen = qb0 * 128, nbk * 128
                    rel = qs - g * 1024
                    nc.tensor.matmul(stg[:, rel:rel + qlen], lhsT_k,
                                     qtile[:, qs:qs + qlen],
                                     start=True, stop=True)
                # one exp op per cluster of runs; split where the masked gap
                # exceeds 512 cols (gap cols cost more than a second op)
                ptg = ptp.tile([128, 1024], F32R, tag="pt", name=f"pt{j}{g}{ik}")
                clusters = []
                for (qb0, nbk, _s) in runs:
                    rlo = qb0 * 128 - g * 1024
                    rhi = rlo + nbk * 128
                    if clusters and rlo - clusters[-1][1] <= 512:
                        clusters[-1][1] = rhi
                    else:
                        clusters.append([rlo, rhi])
                for (clo, chi) in clusters:
                    nc.scalar.activation(ptg[:, clo:chi], stg[:, clo:chi], EXP,
                                         scale=SCALE)
                if gen is not None:
                    gen()
                poll_deferred()
                flush_pending()
                pending[0] = (j, g, ik, runs, ptg, pa_g)

            # ---- lead: qk-gen for pair 0, windows g=0 ----
            for (t, c) in ((0, 0), (1, 0), (0, 1), (1, 1)):
                qk_chunk(0, t, c, lead=True)

            iters = {g: [ik for ik in range(NB) if RUNS[(g, ik)]]
                     for g in range(2)}

            def head_items(j):
                # g=0 fully then g=1: only ONE pa tile live at a time, which
                # frees 2 PSUM banks for a 3-deep stg ring (the exp pipeline)
                phases = [(0, iters[0]), (1, iters[1])]
                return [(g, ik) for (g, iklist) in phases for ik in iklist]

            def edf_schedule(items, chunks):
                """Assign gen chunks to iteration slots by earliest deadline.
                chunks: list of (deadline_slot_inclusive, fn). Returns
                slot -> [fns]; infeasible chunks go to slot 0."""
                slots = {i: [] for i in range(len(items))}
                fill = {i: 0 for i in range(len(items))}
                for dl, fn in sorted(chunks, key=lambda c: c[0]):
                    placed = False
                    # latest-fit: emit just-in-time so gen MMs queue behind
                    # already-arrived DMA data instead of stalling the PE
                    for s in range(min(dl, len(items) - 1), -1, -1):
                        if fill[s] < 2:
                            slots[s].append(fn)
                            fill[s] += 1
                            placed = True
                            break
                    if not placed:
                        slots[0].insert(0, fn)
                return slots

            def head1_chunks(items):
                """v tiles (PV deadline) + pair-0 windows g=1 (QK deadline)."""
                chunks = []
                for m in range(NB):
                    idx = min((i for i, (g, ik) in enumerate(items) if ik == m),
                              default=0)
                    chunks.append((idx + 1, lambda m=m: v_chunk(m)))
                for c in (2, 3):
                    # q chunk c: first QK of window-pair g=c//2 touching it
                    idx = min((i for i, (g, ik) in enumerate(items)
                               if g == c // 2), default=1)
                    chunks.append((max(0, idx - 1),
                                   lambda c=c: qk_chunk(0, 0, c)))
                    # kpad chunk c: first QK with ik in [4c, 4c+4)
                    idx = min((i for i, (g, ik) in enumerate(items)
                               if 4 * c <= ik < 4 * c + 4), default=1)
                    chunks.append((max(0, idx - 1),
                                   lambda c=c: qk_chunk(0, 1, c)))
                return chunks

            def head0_chunks(items):
                """pair-1 gen, needed only by heads 3/2: spread evenly."""
                chunks = []
                pos = 0
                for c in range(4):
                    for t in (0, 1):
                        chunks.append((pos, lambda t=t, c=c: qk_chunk(1, t, c)))
                        pos += 3
                return chunks

            # ---- projection tile emitter (used by proj loop AND as PE
            # filler in head 2's g=1 pass, once g0 windows are final) ----
            projct = [0]

            def emit_proj(m):
                i = projct[0]
                projct[0] += 1
                po = ps.tile([128, D], F32, tag="st", bufs=3, name=f"po{m}")
                for kt in range(2):
                    for c in range(2):
                        nc.tensor.matmul(
                            po[:, c * 512:(c + 1) * 512],
                            attn_t[kt][:, m * 128:(m + 1) * 128],
                            wpr_t[kt][:, c * 512:(c + 1) * 512],
                            start=(kt == 0), stop=(kt == 1))
                ob = pp.tile([128, D], BF16, tag="ob", bufs=3, name=f"ob{m}")
                if i % 2 == 0:
                    nc.vector.tensor_copy(ob[:], po[:])
                else:
                    nc.scalar.copy(ob[:], po[:])
                if m >= 12:
                    # tail tiles: split across both queue engines so the
                    # final drain is half a tile, not a whole one
                    nc.sync.dma_start(out=out_d[m * 128:m * 128 + 64, :],
                                      in_=ob[0:64, :])
                    nc.gpsimd.dma_start(out=out_d[m * 128 + 64:(m + 1) * 128, :],
                                        in_=ob[64:128, :])
                else:
                    deng = nc.sync if i % 2 == 0 else nc.gpsimd
                    deng.dma_start(out=out_d[m * 128:(m + 1) * 128, :],
                                   in_=ob[:])

            for j in (1, 0, 3, 2):
                items = head_items(j)
                if j == 1:
                    genmap = edf_schedule(items, head1_chunks(items))
                elif j == 0:
                    genmap = edf_schedule(items, head0_chunks(items))
                else:
                    genmap = {}
                for w in range(4):
                    first_mm[(j, w)] = True
                cur["stage"] = pp.tile([65, S], F32, tag="stage", bufs=2,
                                       name=f"stage{j}")
                if j % 2 == 1:
                    cur["odd"] = pp.tile([64, S], BF16, tag="odd", bufs=1,
                                         name=f"odd{j}")
                pa = {}
                for i, (g, ik) in enumerate(items):
                    if g not in pa:
                        pa[g] = ps.tile([65, 1024], F32, tag="pa", bufs=1,
                                        name=f"pa{j}{g}")
                    fns = genmap.get(i, [])
                    gen = (lambda fns=fns: [f() for f in fns]) if fns else None
                    attn_iter(j, g, ik, pa[g], gen=gen)
                flush_pending()
                for w in range(4):
                    first_mm.pop((j, w), None)

            # ---- projection + output ----
            # flush all chains except the last head's g=1 (interleaved below)
            force_deferred(keep=(2, 1))
            last_chain = [d for d in deferred if d[2] == (2, 1)]
            deferred.clear()
            for m in range(16):
                if last_chain:
                    last_chain.pop(0)[1]()
                if m == 8:
                    while last_chain:
                        last_chain.pop(0)[1]()
                emit_proj(m)
            while last_chain:
                last_chain.pop(0)[1]()

    # consume first_mm flags at first-visible ik
    nc.compile()
    return nc


def _host_prep(x, w_qkv, w_proj):
    """Per-core input slices, packed 128-partition-major and contiguous.
    x/wqk/wv in bf16, wpr in f32."""
    # xT_sl[b]: [128, 4*8*512] slice-major then k-tile-major
    xT_sl = []
    for b in range(B):
        xT = x[b].T.astype(bfloat16)  # [D, S]
        arr = np.empty((128, 4 * KT * 512), bfloat16)
        for s in range(4):
            for k in range(KT):
                arr[:, s * 4096 + k * 512:s * 4096 + (k + 1) * 512] = \
                    xT[k * 128:(k + 1) * 128, s * 512:(s + 1) * 512]
        xT_sl.append(np.ascontiguousarray(arr))
    in_maps = []
    for c in range(N_CORES):
        b, grp = c // 4, c % 4
        heads = list(range(grp * HPC, (grp + 1) * HPC))
        wqk = np.empty((D, 2 * HPC * HD), np.float32)
        wv = np.zeros((D, VW), np.float32)
        wpr = np.empty((HPC * HD, D), np.float32)
        for j, h in enumerate(heads):
            p, i = j // 2, j % 2  # pair, index in pair
            # pair block: [q_a|q_b][k_a|k_b] at 256*p
            wqk[:, p * 256 + i * HD:p * 256 + (i + 1) * HD] = \
                w_qkv[:, h * HD:(h + 1) * HD]
            wqk[:, p * 256 + 128 + i * HD:p * 256 + 128 + (i + 1) * HD] = \
                w_qkv[:, D + h * HD:D + (h + 1) * HD]
            wv[:, j * (HD + 1):j * (HD + 1) + HD] = \
                w_qkv[:, 2 * D + h * HD:2 * D + (h + 1) * HD]
            wpr[j * HD:(j + 1) * HD, :] = w_proj[h * HD:(h + 1) * HD, :]
        # repack k-tile-major [128, KT*cols]
        wqk_sl = np.empty((128, KT * 512), bfloat16)
        wv_sl = np.empty((128, KT * VW), bfloat16)
        for k in range(KT):
            wqk_sl[:, k * 512:(k + 1) * 512] = \
                wqk[k * 128:(k + 1) * 128, :].astype(bfloat16)
            wv_sl[:, k * VW:(k + 1) * VW] = \
                wv[k * 128:(k + 1) * 128, :].astype(bfloat16)
        in_maps.append({
            "xT": xT_sl[b],
            "wqk": np.ascontiguousarray(wqk_sl),
            "wv": np.ascontiguousarray(wv_sl),
            "wpr": np.ascontiguousarray(wpr).astype(bfloat16),
        })
    return in_maps


def get_program(block_mask: np.ndarray):
    key = np.asarray(block_mask, bool).tobytes()
    if key not in _program_cache:
        _program_cache[key] = _build_program(np.asarray(block_mask, bool))
    return _program_cache[key]


def kernel(x, w_qkv, w_proj, b_proj, block_mask):
    x = np.asarray(x, np.float32)
    w_qkv = np.asarray(w_qkv, np.float32)
    w_proj = np.asarray(w_proj, np.float32)
    b_proj = np.asarray(b_proj, np.float32)
    nc = get_program(block_mask)
    in_maps = _host_prep(x, w_qkv, w_proj)
    res = run_bass_kernel_spmd(nc, in_maps, core_ids=list(range(N_CORES)))
    out = np.empty((B, S, D), np.float32)
    for b in range(B):
        acc = np.asarray(res.results[4 * b]["out"], np.float64)
        for g in range(1, 4):
            acc = acc + np.asarray(res.results[4 * b + g]["out"], np.float64)
        out[b] = (acc + b_proj).astype(np.float32)
    return out
